# revision 7
# baseline (speedup 1.0000x reference)
"""Trainium2 Bass kernel for multi-head attention (B=4, S=1024, D=1024, H=16).

Sharding: 8 cores = batch(4) x head-half(2). Each core projects Q/K/V for its
8 heads over the full 1024 queries/keys of its batch, runs attention, and
computes a PARTIAL output projection (its heads' contribution to all 1024
output columns). The host sums the two bf16 partials per batch in fp32 and
adds bo -- the "all-reduce after the output projection" is a free host-side
pair-sum. This removes the duplicated K/V projections of a query-split
sharding (-25% MACs).

Attention runs in 8 rounds = (q-half 2) x (head-pair 4). Within a round the
two heads of a pair occupy opposite 64-row strips of the PE array (head 2p at
partitions 0:64, head 2p+1 at 64:128), so their K=64 score matmuls execute
CONCURRENTLY via row tiling (tile_position auto-derived from base_partition).
PV matmuls of the previous round's pair interleave between score pairs, and
projection / output-projection groups are spread into the rounds as PE fill
while the exp chain (ScalarE) paces the pipeline.

All rowsum reciprocals go through the DRAM spread ([1,512] -> [128,4]) so the
DVE does 4 lane-parallel elements instead of a 3.3us lane-serial [1,512] op.
Every DMA is HWDGE (SP queue for x/out + per-round traffic, Act queue for
weights); host pretiles all tensors so DMA lines are 8KB contiguous.
"""

import sys

if "/opt/trn_rl_repo" not in sys.path:
    sys.path.insert(0, "/opt/trn_rl_repo")

import numpy as np

B = 4
S = 1024          # sequence (queries and keys)
C = 1024          # d_model
H = 8             # heads per core
D = 64            # head dim
HD = H * D        # 512 features per core
NCORES = 8
SCALE = 0.125     # 1/sqrt(D)

CT = C // 128     # 8 contraction tiles over d_model
JT = HD // 128    # 4 feature tiles
SKT = S // 128    # 8 key tiles
NR = 8            # rounds = 2 q-halves x 4 head pairs

_CACHED = {}


def _emit(tc, ctx):
    import concourse.bass as bass
    from concourse import mybir

    nc = tc.nc
    f32 = mybir.dt.float32
    bf16 = mybir.dt.bfloat16
    Exp = mybir.ActivationFunctionType.Exp
    Copy = mybir.ActivationFunctionType.Copy

    # ---- DRAM I/O (host pretiles everything to [128, ...] contiguous) ----
    xq_d = [
        nc.dram_tensor(f"xq{i}", [128, CT, 512], bf16, kind="ExternalInput").ap()
        for i in range(2)
    ]
    xk_d = [
        nc.dram_tensor(f"xk{i}", [128, CT, 512], bf16, kind="ExternalInput").ap()
        for i in range(2)
    ]
    xv_d = [
        nc.dram_tensor(f"xv{i}", [128, CT, 512], bf16, kind="ExternalInput").ap()
        for i in range(2)
    ]
    wq = nc.dram_tensor("wq", [128, CT, HD], bf16, kind="ExternalInput").ap()
    wk = nc.dram_tensor("wk", [128, CT, HD], bf16, kind="ExternalInput").ap()
    wv = nc.dram_tensor("wv", [128, CT, HD], bf16, kind="ExternalInput").ap()
    wo = nc.dram_tensor("wo", [128, JT, C], bf16, kind="ExternalInput").ap()
    bq = nc.dram_tensor("bq", [128, JT], f32, kind="ExternalInput").ap()
    bk = nc.dram_tensor("bk", [128, JT], f32, kind="ExternalInput").ap()
    bv = nc.dram_tensor("bv", [1, HD], bf16, kind="ExternalInput").ap()
    # out[st] = [128, 1024] rows st*128..st*128+128 of the partial output
    out = nc.dram_tensor("out", [SKT, 128, C], bf16, kind="ExternalOutput").ap()

    # DRAM rows for the rowsum spread/gather around the reciprocal.
    # Row index = unit u = 2*round + parity, u in [0, 16).
    rs_scr = nc.dram_tensor("rs_scr", [2 * NR, 512], f32).ap()
    rr_scr = nc.dram_tensor("rr_scr", [2 * NR, 512], bf16).ap()

    # ---- long-lived SBUF ----
    persist = ctx.enter_context(tc.tile_pool(name="persist", bufs=1))
    qT = persist.tile([128, JT, S], bf16)       # [feat, jt, query]
    kT = persist.tile([128, JT, S], bf16)       # [feat, jt, key]
    v_sb = persist.tile([128, SKT, H, D + 1], bf16)
    wo_sb = persist.tile([128, JT, C], bf16)
    aoT = persist.tile([128, JT, S], bf16)
    xq_sb = persist.tile([128, 2, CT, 512], bf16)   # [p, q-half, ct, q]
    xk_sb = persist.tile([128, 2, CT, 512], bf16)   # [p, k-half, ct, k]
    xv_sb = persist.tile([128, 2, CT, 512], bf16)
    wq_sb = persist.tile([128, CT, HD], bf16)
    wk_sb = persist.tile([128, CT, HD], bf16)
    wv_sb = persist.tile([128, CT, HD], bf16)
    bq_col = persist.tile([128, JT], f32)
    bk_col = persist.tile([128, JT], f32)
    bv_row = persist.tile([1, HD], bf16)
    ones_col = persist.tile([1, 128], bf16)
    ones_p64 = persist.tile([65, 128], bf16)

    nc.vector.memset(ones_col[:, :], 1.0)
    nc.vector.memset(ones_p64[:, :], 1.0)
    nc.vector.memset(v_sb[:, :, :, D : D + 1], 1.0)

    # ---- DMA issue: weights on the Act HWDGE queue, x on the SP queue ----
    nc.scalar.dma_start(out=bq_col[:, :], in_=bq)
    nc.scalar.dma_start(out=bk_col[:, :], in_=bk)
    nc.scalar.dma_start(out=bv_row[:, :], in_=bv)
    nc.scalar.dma_start(out=wq_sb[:, :, :], in_=wq)
    nc.scalar.dma_start(out=wk_sb[:, :, :], in_=wk)
    nc.scalar.dma_start(out=wv_sb[:, :, :], in_=wv)
    nc.scalar.dma_start(out=wo_sb[:, :, :], in_=wo)

    nc.sync.dma_start(out=xq_sb[:, 0], in_=xq_d[0])
    nc.sync.dma_start(out=xk_sb[:, 0], in_=xk_d[0])
    nc.sync.dma_start(out=xk_sb[:, 1], in_=xk_d[1])
    nc.sync.dma_start(out=xv_sb[:, 0], in_=xv_d[0])
    nc.sync.dma_start(out=xv_sb[:, 1], in_=xv_d[1])
    nc.sync.dma_start(out=xq_sb[:, 1], in_=xq_d[1])

    # ---- pools ----
    pj = ctx.enter_context(tc.tile_pool(name="pj_psum", bufs=2, space="PSUM"))
    sp = ctx.enter_context(tc.tile_pool(name="st_psum", bufs=2, space="PSUM"))
    pvp = ctx.enter_context(tc.tile_pool(name="pv_psum", bufs=2, space="PSUM"))
    pt_pool = ctx.enter_context(tc.tile_pool(name="pt", bufs=18))
    of_pool = ctx.enter_context(tc.tile_pool(name="of", bufs=8))
    rrow_pool = ctx.enter_context(tc.tile_pool(name="rrow", bufs=6))
    rsp_pool = ctx.enter_context(tc.tile_pool(name="rsp", bufs=4))
    rrp_pool = ctx.enter_context(tc.tile_pool(name="rrp", bufs=4))
    ao_pool = ctx.enter_context(tc.tile_pool(name="ao_stage", bufs=3))
    out_pool = ctx.enter_context(tc.tile_pool(name="out_sb", bufs=3))

    # ---- projection / output groups (PE fill work) ----
    def proj_q_group(jt, half):
        ps = pj.tile([128, 512], f32, tag="pj")
        for ct in range(CT):
            nc.tensor.matmul(
                ps[:, :],
                lhsT=wq_sb[:, ct, jt * 128 : (jt + 1) * 128],
                rhs=xq_sb[:, half, ct, :],
                start=(ct == 0),
                stop=(ct == CT - 1),
            )
        nc.vector.tensor_scalar_add(
            out=qT[:, jt, half * 512 : (half + 1) * 512],
            in0=ps[:, :],
            scalar1=bq_col[:, jt : jt + 1],
        )

    def proj_k_group(jt, half):
        ps = pj.tile([128, 512], f32, tag="pj")
        for ct in range(CT):
            nc.tensor.matmul(
                ps[:, :],
                lhsT=wk_sb[:, ct, jt * 128 : (jt + 1) * 128],
                rhs=xk_sb[:, half, ct, :],
                start=(ct == 0),
                stop=(ct == CT - 1),
            )
        nc.vector.tensor_scalar_add(
            out=kT[:, jt, half * 512 : (half + 1) * 512],
            in0=ps[:, :],
            scalar1=bk_col[:, jt : jt + 1],
        )

    def proj_v_group(skt):
        kb, ko = skt // 4, skt % 4
        ps = pj.tile([128, 512], f32, tag="pj")
        for ct in range(CT):
            nc.tensor.matmul(
                ps[:, :],
                lhsT=xv_sb[:, kb, ct, ko * 128 : (ko + 1) * 128],
                rhs=wv_sb[:, ct, :],
                start=(ct == 0),
                stop=False,
            )
        nc.tensor.matmul(
            ps[:, :],
            lhsT=ones_col[:, :],
            rhs=bv_row[:, :],
            start=False,
            stop=True,
        )
        nc.vector.tensor_copy(
            out=v_sb[:, skt, :, 0:D],
            in_=ps.rearrange("p (h d) -> p h d", d=D),
        )

    ob_live = {}

    def out_group(st, mb, on_vector=False):
        ps = pj.tile([128, 512], f32, tag="pj")
        for t in range(JT):
            nc.tensor.matmul(
                ps[:, :],
                lhsT=aoT[:, t, st * 128 : (st + 1) * 128],
                rhs=wo_sb[:, t, mb * 512 : (mb + 1) * 512],
                start=(t == 0),
                stop=(t == JT - 1),
            )
        if st not in ob_live:
            o_sb = out_pool.tile([128, C], bf16, tag="ob")
            ob_live[st] = o_sb
        o_sb = ob_live[st]
        sl = slice(mb * 512, (mb + 1) * 512)
        if on_vector:
            nc.vector.tensor_copy(out=o_sb[:, sl], in_=ps[:, :])
        else:
            nc.scalar.activation(out=o_sb[:, sl], in_=ps[:, :], func=Copy)
        if mb == 1:
            nc.sync.dma_start(out=out[st], in_=ob_live.pop(st)[:, :])

    # Per-round fill lists (each entry emits one PSUM group of ~4-9 matmuls).
    fills = {
        0: [
            None,
            None,
            None,
            None,
            lambda: proj_k_group(1, 0),
            lambda: proj_k_group(1, 1),
            lambda: proj_v_group(0),
            lambda: proj_v_group(1),
        ],
        1: [
            lambda: proj_v_group(2),
            lambda: proj_v_group(3),
            lambda: proj_v_group(4),
            lambda: proj_v_group(5),
            lambda: proj_v_group(6),
            lambda: proj_v_group(7),
            lambda: proj_k_group(2, 0),
            lambda: proj_k_group(2, 1),
        ],
        2: [
            lambda: proj_q_group(0, 1),
            lambda: proj_q_group(1, 1),
            lambda: proj_k_group(3, 0),
            lambda: proj_k_group(3, 1),
            None,
            None,
            None,
            None,
        ],
        3: [lambda: proj_q_group(2, 1), None, None, None, None, None, None, None],
        4: [lambda: proj_q_group(3, 1), None, None, None, None, None, None, None],
        6: [
            lambda: out_group(0, 0, on_vector=True),
            None,
            lambda: out_group(0, 1, on_vector=True),
            None,
            lambda: out_group(1, 0, on_vector=True),
            None,
            lambda: out_group(1, 1, on_vector=True),
            None,
        ],
        7: [
            lambda: out_group(2, 0, on_vector=True),
            None,
            lambda: out_group(2, 1, on_vector=True),
            None,
            lambda: out_group(3, 0, on_vector=True),
            None,
            lambda: out_group(3, 1, on_vector=True),
            None,
        ],
    }

    # ---- attention rounds ----
    pt_live = {}      # round -> list of 8 pt tiles
    recip_live = {}   # unit -> (o_f, rsp)
    norm_live = {}    # unit -> (o_f, rrow)

    def finish_pv(r, o_ps, parity):
        u = 2 * r + parity
        o_f = of_pool.tile([65, 512], f32, tag="of")
        nc.vector.tensor_copy(out=o_f[:, :], in_=o_ps[0:65, :])
        nc.sync.dma_start(out=rs_scr[u : u + 1, :], in_=o_f[64:65, :])
        rsp = rsp_pool.tile([128, 4], f32, tag="rsp")
        nc.sync.dma_start(
            out=rsp[:, :], in_=rs_scr[u, :].rearrange("(p q) -> p q", p=128)
        )
        recip_live[u] = (o_f, rsp)

    def emit_recip(r):
        for parity in range(2):
            u = 2 * r + parity
            o_f, rsp = recip_live.pop(u)
            rrow = rrow_pool.tile([65, 512], bf16, tag="rrow")
            rrp = rrp_pool.tile([128, 4], bf16, tag="rrp")
            with nc.allow_low_precision(reason="bf16 rowsum reciprocal"):
                nc.vector.reciprocal(out=rrp[:, :], in_=rsp[:, :])
            nc.sync.dma_start(
                out=rr_scr[u, :].rearrange("(p q) -> p q", p=128), in_=rrp[:, :]
            )
            nc.sync.dma_start(out=rrow[64:65, :], in_=rr_scr[u : u + 1, :])
            norm_live[u] = (o_f, rrow)

    def emit_norm(r):
        qh, p = divmod(r, 4)
        q_sl = slice(qh * 512, (qh + 1) * 512)
        for parity in range(2):
            u = 2 * r + parity
            o_f, rrow = norm_live.pop(u)
            rb = sp.tile([128, 512], f32, tag="st")
            nc.tensor.matmul(
                rb[:, :],
                lhsT=ones_p64[64:65, :],
                rhs=rrow[64:65, :],
                start=True,
                stop=True,
            )
            if parity == 0:
                nc.vector.tensor_mul(
                    out=aoT[0:64, p, q_sl], in0=o_f[0:64, :], in1=rb[0:64, :]
                )
            else:
                ao_stage = ao_pool.tile([64, 512], bf16, tag="ao")
                nc.vector.tensor_mul(
                    out=ao_stage[:, :], in0=o_f[0:64, :], in1=rb[0:64, :]
                )
                nc.sync.dma_start(out=aoT[64:128, p, q_sl], in_=ao_stage[:, :])

    def emit_round(r):
        """Scores for round r's pair; PV of round r-1; lagged recip/norm."""
        qh, p = divmod(r, 4) if r < NR else (None, None)
        prev = r - 1
        o_e = o_o = None
        if 0 <= prev:
            pqh, pp = divmod(prev, 4)
            o_e = pvp.tile([65, 512], f32, tag="pv")
            o_o = pvp.tile([65, 512], f32, tag="pv")
            pv_tiles = pt_live.get(prev)
        rfills = fills.get(r, [None] * 8)
        pt_tiles = []
        for i in range(SKT):
            if i < len(rfills) and rfills[i] is not None:
                rfills[i]()
            if r < NR:
                q_sl = slice(qh * 512, (qh + 1) * 512)
                S_i = sp.tile([128, 2, 512], f32, tag="st")
                nc.tensor.matmul(
                    S_i[:, 0, :],
                    lhsT=kT[0:64, p, i * 128 : (i + 1) * 128],
                    rhs=qT[0:64, p, q_sl],
                    start=True,
                    stop=True,
                )
                nc.tensor.matmul(
                    S_i[:, 1, :],
                    lhsT=kT[64:128, p, i * 128 : (i + 1) * 128],
                    rhs=qT[64:128, p, q_sl],
                    start=True,
                    stop=True,
                )
            if o_e is not None:
                nc.tensor.matmul(
                    o_e[:, :],
                    lhsT=v_sb[:, i, 2 * pp, :],
                    rhs=pv_tiles[i][:, 0, :],
                    start=(i == 0),
                    stop=(i == SKT - 1),
                )
                nc.tensor.matmul(
                    o_o[:, :],
                    lhsT=v_sb[:, i, 2 * pp + 1, :],
                    rhs=pv_tiles[i][:, 1, :],
                    start=(i == 0),
                    stop=(i == SKT - 1),
                )
            if i == 3 and r - 2 >= 0:
                emit_recip(r - 2)
            if r < NR:
                p_t = pt_pool.tile([128, 2, 512], bf16, tag="pt")
                nc.scalar.activation(
                    out=p_t[:, :, :], in_=S_i[:, :, :], func=Exp, scale=SCALE
                )
                pt_tiles.append(p_t)
        if r < NR:
            pt_live[r] = pt_tiles
        if o_e is not None:
            pt_live.pop(prev)
            finish_pv(prev, o_e, 0)
            finish_pv(prev, o_o, 1)
        if r - 2 >= 0:
            emit_norm(r - 2)

    # ---- phase A: Q proj (q-half 0) + K jt0 upfront ----
    for jt in range(JT):
        proj_q_group(jt, 0)
    proj_k_group(0, 0)
    proj_k_group(0, 1)

    # ---- rounds ----
    for r in range(NR):
        emit_round(r)

    # ---- tail: drain PV of round 7, final recip/norm chain, last outs ----
    emit_round(NR)         # pv(7) + recip(6) + norm(6)
    emit_recip(NR - 1)
    emit_norm(NR - 1)
    for st, mb in [(4, 0), (4, 1), (5, 0), (5, 1), (6, 0), (6, 1), (7, 0), (7, 1)]:
        out_group(st, mb)


def _build():
    import concourse.tile as tile
    from concourse import bacc

    from contextlib import ExitStack

    nc = bacc.Bacc(
        "TRN2", target_bir_lowering=False, debug=False, num_devices=NCORES
    )
    with tile.TileContext(nc) as tc:
        with ExitStack() as ctx:
            _emit(tc, ctx)
    nc.compile()
    return nc


def _get_nc():
    if "nc" not in _CACHED:
        _CACHED["nc"] = _build()
    return _CACHED["nc"]


def _chunk(xT, half):
    """[1024, 1024] -> [128, CT, 512] pretile of columns half*512:..."""
    return np.ascontiguousarray(
        xT.reshape(CT, 128, 2, 512)[:, :, half, :].transpose(1, 0, 2)
    )


def _wtile(w):
    """[1024, 512] -> [128, CT, 512]"""
    return np.ascontiguousarray(w.reshape(CT, 128, HD).transpose(1, 0, 2))


def build_in_maps(inputs):
    import ml_dtypes

    bf = ml_dtypes.bfloat16
    f = np.asarray
    queries = f(inputs["queries"], dtype=np.float32)
    keys = f(inputs["keys"], dtype=np.float32)
    values = f(inputs["values"], dtype=np.float32)
    Wq = f(inputs["Wq"], dtype=np.float32)
    Wk = f(inputs["Wk"], dtype=np.float32)
    Wv = f(inputs["Wv"], dtype=np.float32)
    Wo = f(inputs["Wo"], dtype=np.float32)
    bq = f(inputs["bq"], dtype=np.float32)
    bk = f(inputs["bk"], dtype=np.float32)
    bv = f(inputs["bv"], dtype=np.float32)
    in_maps = []
    for c in range(NCORES):
        b, hh = c // 2, c % 2
        cs = slice(hh * HD, (hh + 1) * HD)
        xqT = queries[b].T.astype(bf)
        xkT = keys[b].T.astype(bf)
        xvT = values[b].T.astype(bf)
        wo_c = Wo[cs, :].astype(bf)
        in_maps.append(
            {
                "xq0": _chunk(xqT, 0),
                "xq1": _chunk(xqT, 1),
                "xk0": _chunk(xkT, 0),
                "xk1": _chunk(xkT, 1),
                "xv0": _chunk(xvT, 0),
                "xv1": _chunk(xvT, 1),
                "wq": _wtile(Wq[:, cs].astype(bf)),
                "wk": _wtile(Wk[:, cs].astype(bf)),
                "wv": _wtile(Wv[:, cs].astype(bf)),
                "wo": np.ascontiguousarray(
                    wo_c.reshape(JT, 128, C).transpose(1, 0, 2)
                ),
                "bq": np.ascontiguousarray(bq[cs].reshape(JT, 128).T),
                "bk": np.ascontiguousarray(bk[cs].reshape(JT, 128).T),
                "bv": np.ascontiguousarray(bv[cs].astype(bf).reshape(1, HD)),
            }
        )
    return in_maps


def kernel(**inputs):
    from concourse.bass_utils import run_bass_kernel_spmd

    nc = _get_nc()
    in_maps = build_in_maps(inputs)
    _CACHED["in_maps"] = in_maps
    res = run_bass_kernel_spmd(nc, in_maps, list(range(NCORES)))
    bo = np.asarray(inputs["bo"], dtype=np.float32)
    full = np.empty((B, S, C), dtype=np.float32)
    for b in range(B):
        p0 = res.results[2 * b]["out"].reshape(S, C).astype(np.float32)
        p1 = res.results[2 * b + 1]["out"].reshape(S, C).astype(np.float32)
        full[b] = p0 + p1 + bo
    return full


# revision 12
# speedup vs baseline: 1.0306x; 1.0306x over previous
"""Trainium2 Bass kernel for multi-head attention (B=4, S=1024, D=1024, H=16).

Sharding: 8 cores = batch(4) x head-half(2). Each core projects Q/K/V for its
8 heads over the full 1024 queries/keys of its batch, runs attention, and
computes a PARTIAL output projection (its heads' contribution to all 1024
output columns). The host sums the two bf16 partials per batch in fp32 and
adds bo -- the "all-reduce after the output projection" is a free host-side
pair-sum. This removes the duplicated K/V projections of a query-split
sharding (-25% MACs).

Attention runs in 8 rounds = (q-half 2) x (head-pair 4). Within a round the
two heads of a pair occupy opposite 64-row strips of the PE array (head 2p at
partitions 0:64, head 2p+1 at 64:128), so their K=64 score matmuls execute
CONCURRENTLY via row tiling (tile_position auto-derived from base_partition).
PV matmuls of the previous round's pair interleave between score pairs, and
projection / output-projection groups are spread into the rounds as PE fill
while the exp chain (ScalarE) paces the pipeline.

All rowsum reciprocals go through the DRAM spread ([1,512] -> [128,4]) so the
DVE does 4 lane-parallel elements instead of a 3.3us lane-serial [1,512] op.
Every DMA is HWDGE (SP queue for x/out + per-round traffic, Act queue for
weights); host pretiles all tensors so DMA lines are 8KB contiguous.
"""

import sys

if "/opt/trn_rl_repo" not in sys.path:
    sys.path.insert(0, "/opt/trn_rl_repo")

import numpy as np

B = 4
S = 1024          # sequence (queries and keys)
C = 1024          # d_model
H = 8             # heads per core
D = 64            # head dim
HD = H * D        # 512 features per core
NCORES = 8
SCALE = 0.125     # 1/sqrt(D)

CT = C // 128     # 8 contraction tiles over d_model
JT = HD // 128    # 4 feature tiles
SKT = S // 128    # 8 key tiles
NR = 8            # rounds = 2 q-halves x 4 head pairs

_CACHED = {}


def _emit(tc, ctx):
    import concourse.bass as bass
    from concourse import mybir

    nc = tc.nc
    f32 = mybir.dt.float32
    bf16 = mybir.dt.bfloat16
    Exp = mybir.ActivationFunctionType.Exp
    Copy = mybir.ActivationFunctionType.Copy

    # ---- DRAM I/O (host pretiles everything to [128, ...] contiguous) ----
    xq_d = [
        nc.dram_tensor(f"xq{i}", [128, CT, 512], bf16, kind="ExternalInput").ap()
        for i in range(2)
    ]
    xk_d = [
        nc.dram_tensor(f"xk{i}", [128, CT, 512], bf16, kind="ExternalInput").ap()
        for i in range(2)
    ]
    xv_d = [
        nc.dram_tensor(f"xv{i}", [128, CT, 512], bf16, kind="ExternalInput").ap()
        for i in range(2)
    ]
    wq = nc.dram_tensor("wq", [128, CT, HD], bf16, kind="ExternalInput").ap()
    wk = nc.dram_tensor("wk", [128, CT, HD], bf16, kind="ExternalInput").ap()
    wv = nc.dram_tensor("wv", [128, CT, HD], bf16, kind="ExternalInput").ap()
    wo = nc.dram_tensor("wo", [128, JT, C], bf16, kind="ExternalInput").ap()
    bq = nc.dram_tensor("bq", [128, JT], f32, kind="ExternalInput").ap()
    bk = nc.dram_tensor("bk", [128, JT], f32, kind="ExternalInput").ap()
    bv = nc.dram_tensor("bv", [1, HD], bf16, kind="ExternalInput").ap()
    # out[st] = [128, 1024] rows st*128..st*128+128 of the partial output
    out = nc.dram_tensor("out", [SKT, 128, C], bf16, kind="ExternalOutput").ap()

    # DRAM rows for the rowsum spread/gather around the reciprocal.
    # Row index = unit u = 2*round + parity, u in [0, 16).
    rs_scr = nc.dram_tensor("rs_scr", [2 * NR, 512], f32).ap()
    rr_scr = nc.dram_tensor("rr_scr", [2 * NR, 512], bf16).ap()

    # ---- long-lived SBUF ----
    persist = ctx.enter_context(tc.tile_pool(name="persist", bufs=1))
    qT = persist.tile([128, JT, S], bf16)       # [feat, jt, query]
    kT = persist.tile([128, JT, S], bf16)       # [feat, jt, key]
    v_sb = persist.tile([128, SKT, H, D + 1], bf16)
    wo_sb = persist.tile([128, JT, C], bf16)
    aoT = persist.tile([128, JT, S], bf16)
    xq_sb = persist.tile([128, 2, CT, 512], bf16)   # [p, q-half, ct, q]
    xk_sb = persist.tile([128, 2, CT, 512], bf16)   # [p, k-half, ct, k]
    xv_sb = persist.tile([128, 2, CT, 512], bf16)
    wq_sb = persist.tile([128, CT, HD], bf16)
    wk_sb = persist.tile([128, CT, HD], bf16)
    wv_sb = persist.tile([128, CT, HD], bf16)
    bq_col = persist.tile([128, JT], f32)
    bk_col = persist.tile([128, JT], f32)
    bv_row = persist.tile([1, HD], bf16)
    ones_col = persist.tile([1, 128], bf16)
    ones_p64 = persist.tile([65, 128], bf16)

    nc.vector.memset(ones_col[:, :], 1.0)
    nc.vector.memset(ones_p64[:, :], 1.0)
    nc.vector.memset(v_sb[:, :, :, D : D + 1], 1.0)

    # ---- DMA issue: weights on the Act HWDGE queue, x on the SP queue ----
    # wq first on the (faster-starting) SP queue: the first PE work is the
    # q-half-0 projections and they need wq + xq0 before anything else.
    nc.scalar.dma_start(out=bq_col[:, :], in_=bq)
    nc.scalar.dma_start(out=bk_col[:, :], in_=bk)
    nc.scalar.dma_start(out=bv_row[:, :], in_=bv)
    nc.scalar.dma_start(out=wk_sb[:, :, :], in_=wk)
    nc.scalar.dma_start(out=wv_sb[:, :, :], in_=wv)
    nc.scalar.dma_start(out=wo_sb[:, :, :], in_=wo)

    nc.sync.dma_start(out=wq_sb[:, :, :], in_=wq)
    nc.sync.dma_start(out=xq_sb[:, 0], in_=xq_d[0])
    nc.sync.dma_start(out=xk_sb[:, 0], in_=xk_d[0])
    nc.sync.dma_start(out=xk_sb[:, 1], in_=xk_d[1])
    nc.sync.dma_start(out=xv_sb[:, 0], in_=xv_d[0])
    nc.sync.dma_start(out=xv_sb[:, 1], in_=xv_d[1])
    nc.sync.dma_start(out=xq_sb[:, 1], in_=xq_d[1])

    # ---- pools ----
    pj = ctx.enter_context(tc.tile_pool(name="pj_psum", bufs=2, space="PSUM"))
    sp = ctx.enter_context(tc.tile_pool(name="st_psum", bufs=2, space="PSUM"))
    pvp = ctx.enter_context(tc.tile_pool(name="pv_psum", bufs=2, space="PSUM"))
    pt_pool = ctx.enter_context(tc.tile_pool(name="pt", bufs=18))
    of_pool = ctx.enter_context(tc.tile_pool(name="of", bufs=8))
    rrow_pool = ctx.enter_context(tc.tile_pool(name="rrow", bufs=6))
    rsp_pool = ctx.enter_context(tc.tile_pool(name="rsp", bufs=4))
    rrp_pool = ctx.enter_context(tc.tile_pool(name="rrp", bufs=4))
    ao_pool = ctx.enter_context(tc.tile_pool(name="ao_stage", bufs=3))
    out_pool = ctx.enter_context(tc.tile_pool(name="out_sb", bufs=3))

    # ---- projection / output groups (PE fill work) ----
    def proj_q_group(jt, half):
        ps = pj.tile([128, 512], f32, tag="pj")
        for ct in range(CT):
            nc.tensor.matmul(
                ps[:, :],
                lhsT=wq_sb[:, ct, jt * 128 : (jt + 1) * 128],
                rhs=xq_sb[:, half, ct, :],
                start=(ct == 0),
                stop=(ct == CT - 1),
            )
        nc.vector.tensor_scalar_add(
            out=qT[:, jt, half * 512 : (half + 1) * 512],
            in0=ps[:, :],
            scalar1=bq_col[:, jt : jt + 1],
        )

    def proj_k_group(jt, half):
        ps = pj.tile([128, 512], f32, tag="pj")
        for ct in range(CT):
            nc.tensor.matmul(
                ps[:, :],
                lhsT=wk_sb[:, ct, jt * 128 : (jt + 1) * 128],
                rhs=xk_sb[:, half, ct, :],
                start=(ct == 0),
                stop=(ct == CT - 1),
            )
        nc.vector.tensor_scalar_add(
            out=kT[:, jt, half * 512 : (half + 1) * 512],
            in0=ps[:, :],
            scalar1=bk_col[:, jt : jt + 1],
        )

    def proj_v_group(skt):
        kb, ko = skt // 4, skt % 4
        ps = pj.tile([128, 512], f32, tag="pj")
        for ct in range(CT):
            nc.tensor.matmul(
                ps[:, :],
                lhsT=xv_sb[:, kb, ct, ko * 128 : (ko + 1) * 128],
                rhs=wv_sb[:, ct, :],
                start=(ct == 0),
                stop=False,
            )
        nc.tensor.matmul(
            ps[:, :],
            lhsT=ones_col[:, :],
            rhs=bv_row[:, :],
            start=False,
            stop=True,
        )
        nc.vector.tensor_copy(
            out=v_sb[:, skt, :, 0:D],
            in_=ps.rearrange("p (h d) -> p h d", d=D),
        )

    ob_live = {}

    def out_group(st, mb, on_vector=False):
        ps = pj.tile([128, 512], f32, tag="pj")
        for t in range(JT):
            nc.tensor.matmul(
                ps[:, :],
                lhsT=aoT[:, t, st * 128 : (st + 1) * 128],
                rhs=wo_sb[:, t, mb * 512 : (mb + 1) * 512],
                start=(t == 0),
                stop=(t == JT - 1),
            )
        if st not in ob_live:
            o_sb = out_pool.tile([128, C], bf16, tag="ob")
            ob_live[st] = o_sb
        o_sb = ob_live[st]
        sl = slice(mb * 512, (mb + 1) * 512)
        if on_vector:
            nc.vector.tensor_copy(out=o_sb[:, sl], in_=ps[:, :])
        else:
            nc.scalar.activation(out=o_sb[:, sl], in_=ps[:, :], func=Copy)
        if mb == 1:
            nc.sync.dma_start(out=out[st], in_=ob_live.pop(st)[:, :])

    # Tail out-groups for rows 512+: phase 1 accumulates t0/t1 into an open
    # PSUM group during the PV drain; phase 2 adds t2/t3 once the last norms
    # land. Six groups live in st/pj slots; the last two run as plain groups.
    tail_ps = {}

    def tail_phase1(st, mb, ps):
        tail_ps[(st, mb)] = ps
        for t in (0, 1):
            nc.tensor.matmul(
                ps,
                lhsT=aoT[:, t, st * 128 : (st + 1) * 128],
                rhs=wo_sb[:, t, mb * 512 : (mb + 1) * 512],
                start=(t == 0),
                stop=False,
            )

    def tail_phase2(st, mb, t):
        nc.tensor.matmul(
            tail_ps[(st, mb)],
            lhsT=aoT[:, t, st * 128 : (st + 1) * 128],
            rhs=wo_sb[:, t, mb * 512 : (mb + 1) * 512],
            start=False,
            stop=(t == JT - 1),
        )

    def tail_finish(st, mb, on_vector):
        ps = tail_ps.pop((st, mb))
        if st not in ob_live:
            o_sb = out_pool.tile([128, C], bf16, tag="ob")
            ob_live[st] = o_sb
        o_sb = ob_live[st]
        sl = slice(mb * 512, (mb + 1) * 512)
        if on_vector:
            nc.vector.tensor_copy(out=o_sb[:, sl], in_=ps)
        else:
            nc.scalar.activation(out=o_sb[:, sl], in_=ps, func=Copy)
        if mb == 1:
            nc.sync.dma_start(out=out[st], in_=ob_live.pop(st)[:, :])

    # Per-round fill lists (each entry emits one PSUM group of ~4-9 matmuls).
    fills = {
        0: [
            lambda: proj_q_group(2, 0),
            lambda: proj_q_group(3, 0),
            lambda: proj_k_group(1, 0),
            lambda: proj_k_group(1, 1),
            None,
            None,
            lambda: proj_v_group(0),
            lambda: proj_v_group(1),
        ],
        1: [
            lambda: proj_v_group(2),
            lambda: proj_v_group(3),
            lambda: proj_v_group(4),
            lambda: proj_v_group(5),
            lambda: proj_v_group(6),
            lambda: proj_v_group(7),
            lambda: proj_k_group(2, 0),
            lambda: proj_k_group(2, 1),
        ],
        2: [
            lambda: proj_q_group(0, 1),
            lambda: proj_q_group(1, 1),
            lambda: proj_k_group(3, 0),
            lambda: proj_k_group(3, 1),
            None,
            None,
            None,
            None,
        ],
        3: [lambda: proj_q_group(2, 1), None, None, None, None, None, None, None],
        4: [lambda: proj_q_group(3, 1), None, None, None, None, None, None, None],
        6: [
            lambda: out_group(0, 0, on_vector=True),
            None,
            lambda: out_group(0, 1, on_vector=True),
            None,
            lambda: out_group(1, 0, on_vector=True),
            None,
            lambda: out_group(1, 1, on_vector=True),
            None,
        ],
        7: [
            lambda: out_group(2, 0, on_vector=True),
            None,
            lambda: out_group(2, 1, on_vector=True),
            None,
            lambda: out_group(3, 0, on_vector=True),
            None,
            lambda: out_group(3, 1, on_vector=True),
            None,
        ],
    }

    # ---- attention rounds ----
    pt_live = {}      # round -> list of 8 pt tiles
    pend_of = {}      # unit -> o_f tile awaiting the pair's gather
    recip_live = {}   # round -> (o_f_e, o_f_o, rsp_pair)
    norm_live = {}    # round -> (o_f_e, o_f_o, rrow_pair)

    def _dma_eng(r):
        # Tail rounds issue their recip round-trip from the Act engine so the
        # chain doesn't serialize behind everything on the SP issue queue.
        return nc.scalar if r >= NR - 2 else nc.sync

    def finish_pv(r, o_ps, parity):
        u = 2 * r + parity
        o_f = of_pool.tile([65, 512], f32, tag="of")
        nc.vector.tensor_copy(out=o_f[:, :], in_=o_ps[0:65, :])
        _dma_eng(r).dma_start(out=rs_scr[u : u + 1, :], in_=o_f[64:65, :])
        pend_of[u] = o_f
        if parity == 1:
            rsp = rsp_pool.tile([128, 2, 4], f32, tag="rsp")
            _dma_eng(r).dma_start(
                out=rsp[:, :, :],
                in_=rs_scr[2 * r : 2 * r + 2, :].rearrange(
                    "u (p q) -> p u q", p=128
                ),
            )
            recip_live[r] = (pend_of.pop(2 * r), pend_of.pop(u), rsp)

    def emit_recip(r):
        o_f_e, o_f_o, rsp = recip_live.pop(r)
        rrow = rrow_pool.tile([65, 1024], bf16, tag="rrow")
        rrp = rrp_pool.tile([128, 2, 4], bf16, tag="rrp")
        with nc.allow_low_precision(reason="bf16 rowsum reciprocal"):
            nc.vector.reciprocal(out=rrp[:, :, :], in_=rsp[:, :, :])
        _dma_eng(r).dma_start(
            out=rr_scr[2 * r : 2 * r + 2, :].rearrange("u (p q) -> p u q", p=128),
            in_=rrp[:, :, :],
        )
        _dma_eng(r).dma_start(
            out=rrow[64:65, :], in_=rr_scr[2 * r : 2 * r + 2, :]
        )
        norm_live[r] = (o_f_e, o_f_o, rrow)

    def emit_norm(r, rb_pool=None):
        qh, p = divmod(r, 4)
        q_sl = slice(qh * 512, (qh + 1) * 512)
        o_f_e, o_f_o, rrow = norm_live.pop(r)
        for parity in range(2):
            o_f = o_f_e if parity == 0 else o_f_o
            if rb_pool is None:
                rb = sp.tile([128, 512], f32, tag="st")
            else:
                rb = rb_pool.tile([128, 512], f32, tag="pv")
            nc.tensor.matmul(
                rb[:, :],
                lhsT=ones_p64[64:65, :],
                rhs=rrow[64:65, parity * 512 : (parity + 1) * 512],
                start=True,
                stop=True,
            )
            if parity == 0:
                nc.vector.tensor_mul(
                    out=aoT[0:64, p, q_sl], in0=o_f[0:64, :], in1=rb[0:64, :]
                )
            else:
                ao_stage = ao_pool.tile([64, 512], bf16, tag="ao")
                nc.vector.tensor_mul(
                    out=ao_stage[:, :], in0=o_f[0:64, :], in1=rb[0:64, :]
                )
                nc.sync.dma_start(out=aoT[64:128, p, q_sl], in_=ao_stage[:, :])

    def emit_round(r):
        """Scores for round r's pair; PV of round r-1; lagged recip/norm."""
        qh, p = divmod(r, 4) if r < NR else (None, None)
        prev = r - 1
        o_e = o_o = None
        if 0 <= prev:
            pqh, pp = divmod(prev, 4)
            o_e = pvp.tile([65, 512], f32, tag="pv")
            o_o = pvp.tile([65, 512], f32, tag="pv")
            pv_tiles = pt_live.get(prev)
        rfills = fills.get(r, [None] * 8)
        pt_tiles = []
        for i in range(SKT):
            if i < len(rfills) and rfills[i] is not None:
                rfills[i]()
            if r < NR:
                q_sl = slice(qh * 512, (qh + 1) * 512)
                S_i = sp.tile([128, 2, 512], f32, tag="st")
                nc.tensor.matmul(
                    S_i[:, 0, :],
                    lhsT=kT[0:64, p, i * 128 : (i + 1) * 128],
                    rhs=qT[0:64, p, q_sl],
                    start=True,
                    stop=True,
                )
                nc.tensor.matmul(
                    S_i[:, 1, :],
                    lhsT=kT[64:128, p, i * 128 : (i + 1) * 128],
                    rhs=qT[64:128, p, q_sl],
                    start=True,
                    stop=True,
                )
            if o_e is not None:
                nc.tensor.matmul(
                    o_e[:, :],
                    lhsT=v_sb[:, i, 2 * pp, :],
                    rhs=pv_tiles[i][:, 0, :],
                    start=(i == 0),
                    stop=(i == SKT - 1),
                )
                nc.tensor.matmul(
                    o_o[:, :],
                    lhsT=v_sb[:, i, 2 * pp + 1, :],
                    rhs=pv_tiles[i][:, 1, :],
                    start=(i == 0),
                    stop=(i == SKT - 1),
                )
            if i == 3 and r - 2 >= 0:
                emit_recip(r - 2)
            if r < NR:
                p_t = pt_pool.tile([128, 2, 512], bf16, tag="pt")
                nc.scalar.activation(
                    out=p_t[:, :, :], in_=S_i[:, :, :], func=Exp, scale=SCALE
                )
                pt_tiles.append(p_t)
        if r < NR:
            pt_live[r] = pt_tiles
        if o_e is not None:
            pt_live.pop(prev)
            finish_pv(prev, o_e, 0)
            finish_pv(prev, o_o, 1)
        if r - 2 >= 0:
            emit_norm(r - 2, rb_pool=pvp if r - 2 >= NR - 2 else None)

    # ---- phase A: Q proj jt0/jt1 (q-half 0) + K jt0 upfront ----
    proj_q_group(0, 0)
    proj_q_group(1, 0)
    proj_k_group(0, 0)
    proj_k_group(0, 1)

    # ---- rounds ----
    for r in range(NR):
        emit_round(r)

    # ---- tail ----
    # Open six t0/t1-partial out groups (st/pj slots) before the tail round so
    # their matmuls interleave with the PV(7) drain; only t2/t3 wait on the
    # final norms. st7's two groups run as plain groups at the very end.
    g_st0 = sp.tile([128, 2, 512], f32, tag="st")
    g_st1 = sp.tile([128, 2, 512], f32, tag="st")
    g_pj0 = pj.tile([128, 512], f32, tag="pj")
    g_pj1 = pj.tile([128, 512], f32, tag="pj")
    fills[NR] = [
        lambda: tail_phase1(4, 0, g_st0[:, 0, :]),
        lambda: tail_phase1(4, 1, g_st0[:, 1, :]),
        lambda: tail_phase1(5, 0, g_st1[:, 0, :]),
        lambda: tail_phase1(5, 1, g_st1[:, 1, :]),
        lambda: tail_phase1(6, 0, g_pj0[:, :]),
        lambda: tail_phase1(6, 1, g_pj1[:, :]),
        None,
        None,
    ]
    emit_round(NR)         # pv(7) drain + recip(6) + norm(6) + phase1 fills
    for st, mb in [(4, 0), (4, 1), (5, 0), (5, 1), (6, 0), (6, 1)]:
        tail_phase2(st, mb, 2)
    emit_recip(NR - 1)
    emit_norm(NR - 1, rb_pool=pvp)
    for st, mb in [(4, 0), (4, 1), (5, 0), (5, 1), (6, 0), (6, 1)]:
        tail_phase2(st, mb, 3)
        tail_finish(st, mb, on_vector=(mb == 0))
    out_group(7, 0)
    out_group(7, 1)


def _build():
    import concourse.tile as tile
    from concourse import bacc

    from contextlib import ExitStack

    nc = bacc.Bacc(
        "TRN2", target_bir_lowering=False, debug=False, num_devices=NCORES
    )
    with tile.TileContext(nc) as tc:
        with ExitStack() as ctx:
            _emit(tc, ctx)
    nc.compile()
    return nc


def _get_nc():
    if "nc" not in _CACHED:
        _CACHED["nc"] = _build()
    return _CACHED["nc"]


def _chunk(xT, half):
    """[1024, 1024] -> [128, CT, 512] pretile of columns half*512:..."""
    return np.ascontiguousarray(
        xT.reshape(CT, 128, 2, 512)[:, :, half, :].transpose(1, 0, 2)
    )


def _wtile(w):
    """[1024, 512] -> [128, CT, 512]"""
    return np.ascontiguousarray(w.reshape(CT, 128, HD).transpose(1, 0, 2))


def build_in_maps(inputs):
    import ml_dtypes

    bf = ml_dtypes.bfloat16
    f = np.asarray
    queries = f(inputs["queries"], dtype=np.float32)
    keys = f(inputs["keys"], dtype=np.float32)
    values = f(inputs["values"], dtype=np.float32)
    Wq = f(inputs["Wq"], dtype=np.float32)
    Wk = f(inputs["Wk"], dtype=np.float32)
    Wv = f(inputs["Wv"], dtype=np.float32)
    Wo = f(inputs["Wo"], dtype=np.float32)
    bq = f(inputs["bq"], dtype=np.float32)
    bk = f(inputs["bk"], dtype=np.float32)
    bv = f(inputs["bv"], dtype=np.float32)
    in_maps = []
    for c in range(NCORES):
        b, hh = c // 2, c % 2
        cs = slice(hh * HD, (hh + 1) * HD)
        xqT = queries[b].T.astype(bf)
        xkT = keys[b].T.astype(bf)
        xvT = values[b].T.astype(bf)
        wo_c = Wo[cs, :].astype(bf)
        in_maps.append(
            {
                "xq0": _chunk(xqT, 0),
                "xq1": _chunk(xqT, 1),
                "xk0": _chunk(xkT, 0),
                "xk1": _chunk(xkT, 1),
                "xv0": _chunk(xvT, 0),
                "xv1": _chunk(xvT, 1),
                "wq": _wtile(Wq[:, cs].astype(bf)),
                "wk": _wtile(Wk[:, cs].astype(bf)),
                "wv": _wtile(Wv[:, cs].astype(bf)),
                "wo": np.ascontiguousarray(
                    wo_c.reshape(JT, 128, C).transpose(1, 0, 2)
                ),
                "bq": np.ascontiguousarray(bq[cs].reshape(JT, 128).T),
                "bk": np.ascontiguousarray(bk[cs].reshape(JT, 128).T),
                "bv": np.ascontiguousarray(bv[cs].astype(bf).reshape(1, HD)),
            }
        )
    return in_maps


def kernel(**inputs):
    from concourse.bass_utils import run_bass_kernel_spmd

    nc = _get_nc()
    in_maps = build_in_maps(inputs)
    _CACHED["in_maps"] = in_maps
    res = run_bass_kernel_spmd(nc, in_maps, list(range(NCORES)))
    bo = np.asarray(inputs["bo"], dtype=np.float32)
    full = np.empty((B, S, C), dtype=np.float32)
    for b in range(B):
        p0 = res.results[2 * b]["out"].reshape(S, C).astype(np.float32)
        p1 = res.results[2 * b + 1]["out"].reshape(S, C).astype(np.float32)
        full[b] = p0 + p1 + bo
    return full


# revision 16
# speedup vs baseline: 1.1115x; 1.0786x over previous
"""Trainium2 Bass kernel for multi-head attention (B=4, S=1024, D=1024, H=16).

Sharding: 8 cores = batch(4) x head-half(2). Each core projects Q/K/V for its
8 heads over the full 1024 queries/keys of its batch, runs attention, and
computes a PARTIAL output projection (its heads' contribution to all 1024
output columns). The host sums the two bf16 partials per batch in fp32 and
adds bo -- the "all-reduce after the output projection" is a free host-side
pair-sum. This removes the duplicated K/V projections of a query-split
sharding (-25% MACs).

Attention runs in 8 rounds = (q-half 2) x (head-pair 4). Within a round the
two heads of a pair occupy opposite 64-row strips of the PE array (head 2p at
partitions 0:64, head 2p+1 at 64:128), so their K=64 score matmuls execute
CONCURRENTLY via row tiling (tile_position auto-derived from base_partition).
PV matmuls of the previous round's pair interleave between score pairs, and
projection / output-projection groups are spread into the rounds as PE fill
while the exp chain (ScalarE) paces the pipeline.

All rowsum reciprocals go through the DRAM spread ([1,512] -> [128,4]) so the
DVE does 4 lane-parallel elements instead of a 3.3us lane-serial [1,512] op.
Every DMA is HWDGE (SP queue for x/out + per-round traffic, Act queue for
weights); host pretiles all tensors so DMA lines are 8KB contiguous.
"""

import sys

if "/opt/trn_rl_repo" not in sys.path:
    sys.path.insert(0, "/opt/trn_rl_repo")

import numpy as np

B = 4
S = 1024          # sequence (queries and keys)
C = 1024          # d_model
H = 8             # heads per core
D = 64            # head dim
HD = H * D        # 512 features per core
NCORES = 8
SCALE = 0.125     # 1/sqrt(D)

CT = C // 128     # 8 contraction tiles over d_model
JT = HD // 128    # 4 feature tiles
SKT = S // 128    # 8 key tiles
NR = 8            # rounds = 2 q-halves x 4 head pairs

_CACHED = {}


def _emit(tc, ctx):
    import concourse.bass as bass
    from concourse import mybir

    nc = tc.nc
    f32 = mybir.dt.float32
    bf16 = mybir.dt.bfloat16
    Exp = mybir.ActivationFunctionType.Exp
    Copy = mybir.ActivationFunctionType.Copy

    # ---- DRAM I/O (host pretiles everything to [128, ...] contiguous) ----
    xq_d = [
        nc.dram_tensor(f"xq{i}", [128, CT, 512], bf16, kind="ExternalInput").ap()
        for i in range(2)
    ]
    xk_d = [
        nc.dram_tensor(f"xk{i}", [128, CT, 512], bf16, kind="ExternalInput").ap()
        for i in range(2)
    ]
    xv_d = [
        nc.dram_tensor(f"xv{i}", [128, CT, 512], bf16, kind="ExternalInput").ap()
        for i in range(2)
    ]
    wq = nc.dram_tensor("wq", [128, CT, HD], bf16, kind="ExternalInput").ap()
    wk = nc.dram_tensor("wk", [128, CT, HD], bf16, kind="ExternalInput").ap()
    wv = nc.dram_tensor("wv", [128, CT, HD], bf16, kind="ExternalInput").ap()
    wo = nc.dram_tensor("wo", [128, JT, C], bf16, kind="ExternalInput").ap()
    bq = nc.dram_tensor("bq", [128, JT], f32, kind="ExternalInput").ap()
    bk = nc.dram_tensor("bk", [128, JT], f32, kind="ExternalInput").ap()
    bv = nc.dram_tensor("bv", [1, HD], bf16, kind="ExternalInput").ap()
    # out[st] = [128, 1024] rows st*128..st*128+128 of the partial output
    out = nc.dram_tensor("out", [SKT, 128, C], bf16, kind="ExternalOutput").ap()



    # ---- long-lived SBUF ----
    persist = ctx.enter_context(tc.tile_pool(name="persist", bufs=1))
    qT = persist.tile([128, JT, S], bf16)       # [feat, jt, query]
    kT = persist.tile([128, JT, S], bf16)       # [feat, jt, key]
    v_sb = persist.tile([128, SKT, H, D + 1], bf16)
    wo_sb = persist.tile([128, JT, C], bf16)
    aoT = persist.tile([128, JT, S], bf16)
    xq_sb = persist.tile([128, 2, CT, 512], bf16)   # [p, q-half, ct, q]
    xk_sb = persist.tile([128, 2, CT, 512], bf16)   # [p, k-half, ct, k]
    xv_sb = persist.tile([128, 2, CT, 512], bf16)
    wq_sb = persist.tile([128, CT, HD], bf16)
    wk_sb = persist.tile([128, CT, HD], bf16)
    wv_sb = persist.tile([128, CT, HD], bf16)
    bq_col = persist.tile([128, JT], f32)
    bk_col = persist.tile([128, JT], f32)
    bv_row = persist.tile([1, HD], bf16)
    ones_col = persist.tile([1, 128], bf16)
    ones_p64 = persist.tile([65, 128], bf16)

    nc.vector.memset(ones_col[:, :], 1.0)
    nc.vector.memset(ones_p64[:, :], 1.0)
    nc.vector.memset(v_sb[:, :, :, D : D + 1], 1.0)

    # ---- DMA issue: weights on the Act HWDGE queue, x on the SP queue ----
    # wq first on the (faster-starting) SP queue: the first PE work is the
    # q-half-0 projections and they need wq + xq0 before anything else.
    nc.scalar.dma_start(out=bq_col[:, :], in_=bq)
    nc.scalar.dma_start(out=bk_col[:, :], in_=bk)
    nc.scalar.dma_start(out=bv_row[:, :], in_=bv)
    nc.scalar.dma_start(out=wk_sb[:, :, :], in_=wk)
    nc.scalar.dma_start(out=wv_sb[:, :, :], in_=wv)
    nc.scalar.dma_start(out=wo_sb[:, :, :], in_=wo)

    nc.sync.dma_start(out=wq_sb[:, :, :], in_=wq)
    nc.sync.dma_start(out=xq_sb[:, 0], in_=xq_d[0])
    nc.sync.dma_start(out=xk_sb[:, 0], in_=xk_d[0])
    nc.sync.dma_start(out=xk_sb[:, 1], in_=xk_d[1])
    nc.sync.dma_start(out=xv_sb[:, 0], in_=xv_d[0])
    nc.sync.dma_start(out=xv_sb[:, 1], in_=xv_d[1])
    nc.sync.dma_start(out=xq_sb[:, 1], in_=xq_d[1])

    # ---- pools ----
    pj = ctx.enter_context(tc.tile_pool(name="pj_psum", bufs=2, space="PSUM"))
    sp = ctx.enter_context(tc.tile_pool(name="st_psum", bufs=2, space="PSUM"))
    pvp = ctx.enter_context(tc.tile_pool(name="pv_psum", bufs=2, space="PSUM"))
    pt_pool = ctx.enter_context(tc.tile_pool(name="pt", bufs=18))
    of_pool = ctx.enter_context(tc.tile_pool(name="of", bufs=8))
    rrow_pool = ctx.enter_context(tc.tile_pool(name="rrow", bufs=6))
    rsp_pool = ctx.enter_context(tc.tile_pool(name="rsp", bufs=4))
    rrp_pool = ctx.enter_context(tc.tile_pool(name="rrp", bufs=4))
    ao_pool = ctx.enter_context(tc.tile_pool(name="ao_stage", bufs=3))
    out_pool = ctx.enter_context(tc.tile_pool(name="out_sb", bufs=3))

    # ---- projection / output groups (PE fill work) ----
    def proj_q_group(jt, half):
        ps = pj.tile([128, 512], f32, tag="pj")
        for ct in range(CT):
            nc.tensor.matmul(
                ps[:, :],
                lhsT=wq_sb[:, ct, jt * 128 : (jt + 1) * 128],
                rhs=xq_sb[:, half, ct, :],
                start=(ct == 0),
                stop=(ct == CT - 1),
            )
        nc.vector.tensor_scalar_add(
            out=qT[:, jt, half * 512 : (half + 1) * 512],
            in0=ps[:, :],
            scalar1=bq_col[:, jt : jt + 1],
        )

    def proj_k_group(jt, half):
        ps = pj.tile([128, 512], f32, tag="pj")
        for ct in range(CT):
            nc.tensor.matmul(
                ps[:, :],
                lhsT=wk_sb[:, ct, jt * 128 : (jt + 1) * 128],
                rhs=xk_sb[:, half, ct, :],
                start=(ct == 0),
                stop=(ct == CT - 1),
            )
        nc.vector.tensor_scalar_add(
            out=kT[:, jt, half * 512 : (half + 1) * 512],
            in0=ps[:, :],
            scalar1=bk_col[:, jt : jt + 1],
        )

    def proj_v_group(skt):
        kb, ko = skt // 4, skt % 4
        ps = pj.tile([128, 512], f32, tag="pj")
        for ct in range(CT):
            nc.tensor.matmul(
                ps[:, :],
                lhsT=xv_sb[:, kb, ct, ko * 128 : (ko + 1) * 128],
                rhs=wv_sb[:, ct, :],
                start=(ct == 0),
                stop=False,
            )
        nc.tensor.matmul(
            ps[:, :],
            lhsT=ones_col[:, :],
            rhs=bv_row[:, :],
            start=False,
            stop=True,
        )
        nc.vector.tensor_copy(
            out=v_sb[:, skt, :, 0:D],
            in_=ps.rearrange("p (h d) -> p h d", d=D),
        )

    ob_live = {}

    def out_group(st, mb, on_vector=False):
        ps = pj.tile([128, 512], f32, tag="pj")
        for t in range(JT):
            nc.tensor.matmul(
                ps[:, :],
                lhsT=aoT[:, t, st * 128 : (st + 1) * 128],
                rhs=wo_sb[:, t, mb * 512 : (mb + 1) * 512],
                start=(t == 0),
                stop=(t == JT - 1),
            )
        if st not in ob_live:
            o_sb = out_pool.tile([128, C], bf16, tag="ob")
            ob_live[st] = o_sb
        o_sb = ob_live[st]
        sl = slice(mb * 512, (mb + 1) * 512)
        if on_vector:
            nc.vector.tensor_copy(out=o_sb[:, sl], in_=ps[:, :])
        else:
            nc.scalar.activation(out=o_sb[:, sl], in_=ps[:, :], func=Copy)
        if mb == 1:
            nc.sync.dma_start(out=out[st], in_=ob_live.pop(st)[:, :])

    # Tail out-groups for rows 512+: phase 1 accumulates t0/t1 into an open
    # PSUM group during the PV drain; phase 2 adds t2/t3 once the last norms
    # land. Six groups live in st/pj slots; the last two run as plain groups.
    tail_ps = {}

    def tail_phase1(st, mb, ps):
        tail_ps[(st, mb)] = ps
        for t in (0, 1):
            nc.tensor.matmul(
                ps,
                lhsT=aoT[:, t, st * 128 : (st + 1) * 128],
                rhs=wo_sb[:, t, mb * 512 : (mb + 1) * 512],
                start=(t == 0),
                stop=False,
            )

    def tail_phase2(st, mb, t):
        nc.tensor.matmul(
            tail_ps[(st, mb)],
            lhsT=aoT[:, t, st * 128 : (st + 1) * 128],
            rhs=wo_sb[:, t, mb * 512 : (mb + 1) * 512],
            start=False,
            stop=(t == JT - 1),
        )

    def tail_finish(st, mb, on_vector):
        ps = tail_ps.pop((st, mb))
        if st not in ob_live:
            o_sb = out_pool.tile([128, C], bf16, tag="ob")
            ob_live[st] = o_sb
        o_sb = ob_live[st]
        sl = slice(mb * 512, (mb + 1) * 512)
        if on_vector:
            nc.vector.tensor_copy(out=o_sb[:, sl], in_=ps)
        else:
            nc.scalar.activation(out=o_sb[:, sl], in_=ps, func=Copy)
        if mb == 1:
            nc.sync.dma_start(out=out[st], in_=ob_live.pop(st)[:, :])

    # Per-round fill lists (each entry emits one PSUM group of ~4-9 matmuls).
    fills = {
        0: [
            lambda: proj_k_group(1, 0),
            lambda: proj_k_group(1, 1),
            lambda: proj_q_group(2, 0),
            None,
            lambda: proj_v_group(0),
            lambda: proj_v_group(1),
            lambda: proj_v_group(2),
            lambda: proj_v_group(3),
        ],
        1: [
            lambda: proj_v_group(4),
            lambda: proj_v_group(5),
            lambda: proj_v_group(6),
            lambda: proj_v_group(7),
            lambda: proj_k_group(2, 0),
            lambda: proj_k_group(2, 1),
            lambda: proj_q_group(3, 0),
            None,
        ],
        2: [
            lambda: proj_k_group(3, 0),
            lambda: proj_k_group(3, 1),
            None,
            None,
            None,
            None,
            None,
            None,
        ],
        3: [lambda: proj_q_group(0, 1), lambda: proj_q_group(1, 1),
            None, None, None, None, None, None],
        4: [lambda: proj_q_group(2, 1), None, None, None, None, None, None, None],
        5: [lambda: proj_q_group(3, 1), None, None, None, None, None, None, None],
        6: [
            lambda: out_group(0, 0, on_vector=True),
            None,
            lambda: out_group(0, 1, on_vector=True),
            None,
            lambda: out_group(1, 0, on_vector=True),
            None,
            lambda: out_group(1, 1, on_vector=True),
            None,
        ],
        7: [
            lambda: out_group(2, 0, on_vector=True),
            None,
            lambda: out_group(2, 1, on_vector=True),
            None,
            lambda: out_group(3, 0, on_vector=True),
            None,
            lambda: out_group(3, 1, on_vector=True),
            None,
        ],
    }

    # ---- attention rounds ----
    pt_live = {}      # round -> list of 8 pt tiles
    pend_of = {}      # unit -> o_f tile awaiting the pair's gather
    recip_live = {}   # round -> (o_f_e, o_f_o, rsp_pair)
    norm_live = {}    # round -> (o_f_e, o_f_o, rrow_pair)

    def _dma_eng(r):
        # Tail rounds issue their recip round-trip from the Act engine so the
        # chain doesn't serialize behind everything on the SP issue queue.
        return nc.scalar if r >= NR - 2 else nc.sync

    def finish_pv(r, o_ps, parity):
        # Rowsum row [1,512] -> [128,4] lane spread as ONE direct SBUF->SBUF
        # reshape DMA (no DRAM round trip: each hop costs ~2us of completion
        # latency, which is exposed on the tail's critical path).
        u = 2 * r + parity
        o_f = of_pool.tile([65, 512], f32, tag="of")
        nc.vector.tensor_copy(out=o_f[:, :], in_=o_ps[0:65, :])
        pend_of[u] = o_f
        if parity == 0:
            rsp = rsp_pool.tile([128, 2, 4], f32, tag="rsp")
            pend_of["rsp"] = rsp
        else:
            rsp = pend_of.pop("rsp")
        _dma_eng(r).dma_start(out=rsp[:, parity, :], in_=o_f[64:65, :])
        if parity == 1:
            recip_live[r] = (pend_of.pop(2 * r), pend_of.pop(u), rsp)

    def emit_recip(r):
        o_f_e, o_f_o, rsp = recip_live.pop(r)
        rrow = rrow_pool.tile([65, 1024], bf16, tag="rrow")
        rrp = rrp_pool.tile([128, 2, 4], bf16, tag="rrp")
        with nc.allow_low_precision(reason="bf16 rowsum reciprocal"):
            nc.vector.reciprocal(out=rrp[:, :, :], in_=rsp[:, :, :])
        for u in range(2):
            _dma_eng(r).dma_start(
                out=rrow[64:65, u * 512 : (u + 1) * 512], in_=rrp[:, u, :]
            )
        norm_live[r] = (o_f_e, o_f_o, rrow)

    def emit_norm(r, rb_pool=None):
        qh, p = divmod(r, 4)
        q_sl = slice(qh * 512, (qh + 1) * 512)
        o_f_e, o_f_o, rrow = norm_live.pop(r)
        for parity in range(2):
            o_f = o_f_e if parity == 0 else o_f_o
            if rb_pool is None:
                rb = sp.tile([128, 512], f32, tag="st")
            else:
                rb = rb_pool.tile([128, 512], f32, tag="pv")
            nc.tensor.matmul(
                rb[:, :],
                lhsT=ones_p64[64:65, :],
                rhs=rrow[64:65, parity * 512 : (parity + 1) * 512],
                start=True,
                stop=True,
            )
            if parity == 0:
                nc.vector.tensor_mul(
                    out=aoT[0:64, p, q_sl], in0=o_f[0:64, :], in1=rb[0:64, :]
                )
            else:
                ao_stage = ao_pool.tile([64, 512], bf16, tag="ao")
                nc.vector.tensor_mul(
                    out=ao_stage[:, :], in0=o_f[0:64, :], in1=rb[0:64, :]
                )
                nc.sync.dma_start(out=aoT[64:128, p, q_sl], in_=ao_stage[:, :])

    def emit_round(r):
        """Scores for round r's pair; PV of round r-1; lagged recip/norm."""
        qh, p = divmod(r, 4) if r < NR else (None, None)
        prev = r - 1
        o_e = o_o = None
        if 0 <= prev:
            pqh, pp = divmod(prev, 4)
            o_e = pvp.tile([65, 512], f32, tag="pv")
            o_o = pvp.tile([65, 512], f32, tag="pv")
            pv_tiles = pt_live.get(prev)
        rfills = fills.get(r, [None] * 8)
        pt_tiles = []
        for i in range(SKT):
            if i < len(rfills) and rfills[i] is not None:
                rfills[i]()
            if r < NR:
                q_sl = slice(qh * 512, (qh + 1) * 512)
                S_i = sp.tile([128, 2, 512], f32, tag="st")
                nc.tensor.matmul(
                    S_i[:, 0, :],
                    lhsT=kT[0:64, p, i * 128 : (i + 1) * 128],
                    rhs=qT[0:64, p, q_sl],
                    start=True,
                    stop=True,
                )
                nc.tensor.matmul(
                    S_i[:, 1, :],
                    lhsT=kT[64:128, p, i * 128 : (i + 1) * 128],
                    rhs=qT[64:128, p, q_sl],
                    start=True,
                    stop=True,
                )
            if o_e is not None:
                nc.tensor.matmul(
                    o_e[:, :],
                    lhsT=v_sb[:, i, 2 * pp, :],
                    rhs=pv_tiles[i][:, 0, :],
                    start=(i == 0),
                    stop=(i == SKT - 1),
                )
                nc.tensor.matmul(
                    o_o[:, :],
                    lhsT=v_sb[:, i, 2 * pp + 1, :],
                    rhs=pv_tiles[i][:, 1, :],
                    start=(i == 0),
                    stop=(i == SKT - 1),
                )
            if i == 3 and r - 2 >= 0:
                emit_recip(r - 2)
            if r < NR:
                p_t = pt_pool.tile([128, 2, 512], bf16, tag="pt")
                nc.scalar.activation(
                    out=p_t[:, :, :], in_=S_i[:, :, :], func=Exp, scale=SCALE
                )
                pt_tiles.append(p_t)
        if r < NR:
            pt_live[r] = pt_tiles
        if o_e is not None:
            pt_live.pop(prev)
            finish_pv(prev, o_e, 0)
            finish_pv(prev, o_o, 1)
        if r - 2 >= 0:
            emit_norm(r - 2, rb_pool=pvp if r - 2 >= NR - 2 else None)

    # ---- phase A: Q proj jt0/jt1 (q-half 0) + K jt0 upfront ----
    proj_q_group(0, 0)
    proj_q_group(1, 0)
    proj_k_group(0, 0)
    proj_k_group(0, 1)

    # ---- rounds ----
    for r in range(NR):
        emit_round(r)

    # ---- tail ----
    # Open six t0/t1-partial out groups (st/pj slots) before the tail round so
    # their matmuls interleave with the PV(7) drain; only t2/t3 wait on the
    # final norms. st7's two groups run as plain groups at the very end.
    g_st0 = sp.tile([128, 2, 512], f32, tag="st")
    g_st1 = sp.tile([128, 2, 512], f32, tag="st")
    g_pj0 = pj.tile([128, 512], f32, tag="pj")
    g_pj1 = pj.tile([128, 512], f32, tag="pj")
    fills[NR] = [
        lambda: tail_phase1(4, 0, g_st0[:, 0, :]),
        lambda: tail_phase1(4, 1, g_st0[:, 1, :]),
        lambda: tail_phase1(5, 0, g_st1[:, 0, :]),
        lambda: tail_phase1(5, 1, g_st1[:, 1, :]),
        lambda: tail_phase1(6, 0, g_pj0[:, :]),
        lambda: tail_phase1(6, 1, g_pj1[:, :]),
        None,
        None,
    ]
    emit_round(NR)         # pv(7) drain + recip(6) + norm(6) + phase1 fills
    for st, mb in [(4, 0), (4, 1), (5, 0), (5, 1), (6, 0), (6, 1)]:
        tail_phase2(st, mb, 2)
    emit_recip(NR - 1)
    emit_norm(NR - 1, rb_pool=pvp)
    for st, mb in [(4, 0), (4, 1), (5, 0), (5, 1), (6, 0), (6, 1)]:
        tail_phase2(st, mb, 3)
        tail_finish(st, mb, on_vector=(mb == 0))
    out_group(7, 0)
    out_group(7, 1)


def _build():
    import concourse.tile as tile
    from concourse import bacc

    from contextlib import ExitStack

    nc = bacc.Bacc(
        "TRN2", target_bir_lowering=False, debug=False, num_devices=NCORES
    )
    with tile.TileContext(nc) as tc:
        with ExitStack() as ctx:
            _emit(tc, ctx)
    nc.compile()
    return nc


def _get_nc():
    if "nc" not in _CACHED:
        _CACHED["nc"] = _build()
    return _CACHED["nc"]


def _chunk(xT, half):
    """[1024, 1024] -> [128, CT, 512] pretile of columns half*512:..."""
    return np.ascontiguousarray(
        xT.reshape(CT, 128, 2, 512)[:, :, half, :].transpose(1, 0, 2)
    )


def _wtile(w):
    """[1024, 512] -> [128, CT, 512]"""
    return np.ascontiguousarray(w.reshape(CT, 128, HD).transpose(1, 0, 2))


def build_in_maps(inputs):
    import ml_dtypes

    bf = ml_dtypes.bfloat16
    f = np.asarray
    queries = f(inputs["queries"], dtype=np.float32)
    keys = f(inputs["keys"], dtype=np.float32)
    values = f(inputs["values"], dtype=np.float32)
    Wq = f(inputs["Wq"], dtype=np.float32)
    Wk = f(inputs["Wk"], dtype=np.float32)
    Wv = f(inputs["Wv"], dtype=np.float32)
    Wo = f(inputs["Wo"], dtype=np.float32)
    bq = f(inputs["bq"], dtype=np.float32)
    bk = f(inputs["bk"], dtype=np.float32)
    bv = f(inputs["bv"], dtype=np.float32)
    in_maps = []
    for c in range(NCORES):
        b, hh = c // 2, c % 2
        cs = slice(hh * HD, (hh + 1) * HD)
        xqT = queries[b].T.astype(bf)
        xkT = keys[b].T.astype(bf)
        xvT = values[b].T.astype(bf)
        wo_c = Wo[cs, :].astype(bf)
        in_maps.append(
            {
                "xq0": _chunk(xqT, 0),
                "xq1": _chunk(xqT, 1),
                "xk0": _chunk(xkT, 0),
                "xk1": _chunk(xkT, 1),
                "xv0": _chunk(xvT, 0),
                "xv1": _chunk(xvT, 1),
                "wq": _wtile(Wq[:, cs].astype(bf)),
                "wk": _wtile(Wk[:, cs].astype(bf)),
                "wv": _wtile(Wv[:, cs].astype(bf)),
                "wo": np.ascontiguousarray(
                    wo_c.reshape(JT, 128, C).transpose(1, 0, 2)
                ),
                "bq": np.ascontiguousarray(bq[cs].reshape(JT, 128).T),
                "bk": np.ascontiguousarray(bk[cs].reshape(JT, 128).T),
                "bv": np.ascontiguousarray(bv[cs].astype(bf).reshape(1, HD)),
            }
        )
    return in_maps


def kernel(**inputs):
    from concourse.bass_utils import run_bass_kernel_spmd

    nc = _get_nc()
    in_maps = build_in_maps(inputs)
    _CACHED["in_maps"] = in_maps
    res = run_bass_kernel_spmd(nc, in_maps, list(range(NCORES)))
    bo = np.asarray(inputs["bo"], dtype=np.float32)
    full = np.empty((B, S, C), dtype=np.float32)
    for b in range(B):
        p0 = res.results[2 * b]["out"].reshape(S, C).astype(np.float32)
        p1 = res.results[2 * b + 1]["out"].reshape(S, C).astype(np.float32)
        full[b] = p0 + p1 + bo
    return full


# revision 23
# speedup vs baseline: 1.1349x; 1.0210x over previous
"""Trainium2 Bass kernel for multi-head attention (B=4, S=1024, D=1024, H=16).

Sharding: 8 cores = batch(4) x head-half(2). Each core projects Q/K/V for its
8 heads over the full 1024 queries/keys of its batch, runs attention, and
computes a PARTIAL output projection (its heads' contribution to all 1024
output columns). The host sums the two bf16 partials per batch in fp32 and
adds bo -- the "all-reduce after the output projection" is a free host-side
pair-sum. This removes the duplicated K/V projections of a query-split
sharding (-25% MACs).

Attention runs in 8 rounds = (q-half 2) x (head-pair 4). Within a round the
two heads of a pair occupy opposite 64-row strips of the PE array (head 2p at
partitions 0:64, head 2p+1 at 64:128), so their K=64 score matmuls execute
CONCURRENTLY via row tiling (tile_position auto-derived from base_partition).
PV matmuls of the previous round's pair interleave between score pairs, and
projection / output-projection groups are spread into the rounds as PE fill
while the exp chain (ScalarE) paces the pipeline.

All rowsum reciprocals go through the DRAM spread ([1,512] -> [128,4]) so the
DVE does 4 lane-parallel elements instead of a 3.3us lane-serial [1,512] op.
Every DMA is HWDGE (SP queue for x/out + per-round traffic, Act queue for
weights); host pretiles all tensors so DMA lines are 8KB contiguous.
"""

import sys

if "/opt/trn_rl_repo" not in sys.path:
    sys.path.insert(0, "/opt/trn_rl_repo")

import numpy as np

B = 4
S = 1024          # sequence (queries and keys)
C = 1024          # d_model
H = 8             # heads per core
D = 64            # head dim
HD = H * D        # 512 features per core
NCORES = 8
SCALE = 0.125     # 1/sqrt(D)

CT = C // 128     # 8 contraction tiles over d_model
JT = HD // 128    # 4 feature tiles
SKT = S // 128    # 8 key tiles
NR = 8            # rounds = 2 q-halves x 4 head pairs

_CACHED = {}


def _emit(tc, ctx):
    import concourse.bass as bass
    from concourse import mybir

    nc = tc.nc
    f32 = mybir.dt.float32
    bf16 = mybir.dt.bfloat16
    Exp = mybir.ActivationFunctionType.Exp
    Copy = mybir.ActivationFunctionType.Copy

    # ---- DRAM I/O (host pretiles everything to [128, ...] contiguous) ----
    xq_d = [
        nc.dram_tensor(f"xq{i}", [128, CT, 512], bf16, kind="ExternalInput").ap()
        for i in range(2)
    ]
    xk_d = [
        nc.dram_tensor(f"xk{i}", [128, CT, 512], bf16, kind="ExternalInput").ap()
        for i in range(2)
    ]
    xv_d = [
        nc.dram_tensor(f"xv{i}", [128, CT, 512], bf16, kind="ExternalInput").ap()
        for i in range(2)
    ]
    wq = nc.dram_tensor("wq", [128, CT, HD], bf16, kind="ExternalInput").ap()
    wk = nc.dram_tensor("wk", [128, CT, HD], bf16, kind="ExternalInput").ap()
    wv = nc.dram_tensor("wv", [128, CT, HD], bf16, kind="ExternalInput").ap()
    wo = nc.dram_tensor("wo", [128, JT, C], bf16, kind="ExternalInput").ap()
    bq = nc.dram_tensor("bq", [128, JT], f32, kind="ExternalInput").ap()
    bk = nc.dram_tensor("bk", [128, JT], f32, kind="ExternalInput").ap()
    bv = nc.dram_tensor("bv", [1, HD], bf16, kind="ExternalInput").ap()
    # out[st] = [128, 1024] rows st*128..st*128+128 of the partial output
    out = nc.dram_tensor("out", [SKT, 128, C], bf16, kind="ExternalOutput").ap()



    # ---- long-lived SBUF ----
    persist = ctx.enter_context(tc.tile_pool(name="persist", bufs=1))
    qT = persist.tile([128, JT, S], bf16)       # [feat, jt, query]
    kT = persist.tile([128, JT, S], bf16)       # [feat, jt, key]
    v_sb = persist.tile([128, SKT, H, D + 1], bf16)
    wo_sb = persist.tile([128, JT, C], bf16)
    aoT = persist.tile([128, JT, S], bf16)
    xq_sb = persist.tile([128, 2, CT, 512], bf16)   # [p, q-half, ct, q]
    xk_sb = persist.tile([128, 2, CT, 512], bf16)   # [p, k-half, ct, k]
    xv_sb = persist.tile([128, 2, CT, 512], bf16)
    wq_sb = persist.tile([128, CT, HD], bf16)
    wk_sb = persist.tile([128, CT, HD], bf16)
    wv_sb = persist.tile([128, CT, HD], bf16)
    bq_col = persist.tile([128, JT], f32)
    bk_col = persist.tile([128, JT], f32)
    bv_row = persist.tile([1, HD], bf16)
    ones_col = persist.tile([1, 128], bf16)
    ones_p64 = persist.tile([65, 128], bf16)

    nc.vector.memset(ones_col[:, :], 1.0)
    nc.vector.memset(ones_p64[:, :], 1.0)
    nc.vector.memset(v_sb[:, :, :, D : D + 1], 1.0)

    # ---- DMA issue: weights on the Act HWDGE queue, x on the SP queue ----
    # wq first on the (faster-starting) SP queue: the first PE work is the
    # q-half-0 projections and they need wq + xq0 before anything else.
    nc.scalar.dma_start(out=bq_col[:, :], in_=bq)
    nc.scalar.dma_start(out=bk_col[:, :], in_=bk)
    nc.scalar.dma_start(out=bv_row[:, :], in_=bv)
    nc.scalar.dma_start(out=wk_sb[:, :, :], in_=wk)
    nc.scalar.dma_start(out=wv_sb[:, :, :], in_=wv)
    nc.scalar.dma_start(out=wo_sb[:, :, :], in_=wo)

    nc.sync.dma_start(out=wq_sb[:, :, :], in_=wq)
    nc.sync.dma_start(out=xq_sb[:, 0], in_=xq_d[0])
    nc.sync.dma_start(out=xk_sb[:, 0], in_=xk_d[0])
    nc.sync.dma_start(out=xk_sb[:, 1], in_=xk_d[1])
    nc.sync.dma_start(out=xv_sb[:, 0], in_=xv_d[0])
    nc.sync.dma_start(out=xv_sb[:, 1], in_=xv_d[1])
    nc.sync.dma_start(out=xq_sb[:, 1], in_=xq_d[1])

    # ---- pools ----
    pj = ctx.enter_context(tc.tile_pool(name="pj_psum", bufs=2, space="PSUM"))
    sp = ctx.enter_context(tc.tile_pool(name="st_psum", bufs=2, space="PSUM"))
    pvp = ctx.enter_context(tc.tile_pool(name="pv_psum", bufs=2, space="PSUM"))
    pt_pool = ctx.enter_context(tc.tile_pool(name="pt", bufs=18))
    of_pool = ctx.enter_context(tc.tile_pool(name="of", bufs=8))
    rrow_pool = ctx.enter_context(tc.tile_pool(name="rrow", bufs=6))
    rsp_pool = ctx.enter_context(tc.tile_pool(name="rsp", bufs=4))
    rrp_pool = ctx.enter_context(tc.tile_pool(name="rrp", bufs=4))
    ao_pool = ctx.enter_context(tc.tile_pool(name="ao_stage", bufs=3))
    out_pool = ctx.enter_context(tc.tile_pool(name="out_sb", bufs=3))

    # ---- projection / output groups (PE fill work) ----
    def proj_q_group(jt, half):
        ps = pj.tile([128, 512], f32, tag="pj")
        for ct in range(CT):
            nc.tensor.matmul(
                ps[:, :],
                lhsT=wq_sb[:, ct, jt * 128 : (jt + 1) * 128],
                rhs=xq_sb[:, half, ct, :],
                start=(ct == 0),
                stop=(ct == CT - 1),
            )
        nc.vector.tensor_scalar_add(
            out=qT[:, jt, half * 512 : (half + 1) * 512],
            in0=ps[:, :],
            scalar1=bq_col[:, jt : jt + 1],
        )

    def proj_k_group(jt, half):
        ps = pj.tile([128, 512], f32, tag="pj")
        for ct in range(CT):
            nc.tensor.matmul(
                ps[:, :],
                lhsT=wk_sb[:, ct, jt * 128 : (jt + 1) * 128],
                rhs=xk_sb[:, half, ct, :],
                start=(ct == 0),
                stop=(ct == CT - 1),
            )
        nc.vector.tensor_scalar_add(
            out=kT[:, jt, half * 512 : (half + 1) * 512],
            in0=ps[:, :],
            scalar1=bk_col[:, jt : jt + 1],
        )

    def proj_v_group(skt):
        kb, ko = skt // 4, skt % 4
        ps = pj.tile([128, 512], f32, tag="pj")
        for ct in range(CT):
            nc.tensor.matmul(
                ps[:, :],
                lhsT=xv_sb[:, kb, ct, ko * 128 : (ko + 1) * 128],
                rhs=wv_sb[:, ct, :],
                start=(ct == 0),
                stop=False,
            )
        nc.tensor.matmul(
            ps[:, :],
            lhsT=ones_col[:, :],
            rhs=bv_row[:, :],
            start=False,
            stop=True,
        )
        nc.vector.tensor_copy(
            out=v_sb[:, skt, :, 0:D],
            in_=ps.rearrange("p (h d) -> p h d", d=D),
        )

    ob_live = {}

    def out_group(st, mb, on_vector=False):
        ps = pj.tile([128, 512], f32, tag="pj")
        for t in range(JT):
            nc.tensor.matmul(
                ps[:, :],
                lhsT=aoT[:, t, st * 128 : (st + 1) * 128],
                rhs=wo_sb[:, t, mb * 512 : (mb + 1) * 512],
                start=(t == 0),
                stop=(t == JT - 1),
            )
        if st not in ob_live:
            o_sb = out_pool.tile([128, C], bf16, tag="ob")
            ob_live[st] = o_sb
        o_sb = ob_live[st]
        sl = slice(mb * 512, (mb + 1) * 512)
        if on_vector:
            nc.vector.tensor_copy(out=o_sb[:, sl], in_=ps[:, :])
        else:
            nc.scalar.activation(out=o_sb[:, sl], in_=ps[:, :], func=Copy)
        if mb == 1:
            nc.sync.dma_start(out=out[st], in_=ob_live.pop(st)[:, :])

    # Tail out-groups for rows 512+: phase 1 accumulates t0/t1 into an open
    # PSUM group during the PV drain; phase 2 adds t2/t3 once the last norms
    # land. Six groups live in st/pj slots; the last two run as plain groups.
    tail_ps = {}

    def tail_phase1(st, mb, ps):
        tail_ps[(st, mb)] = ps
        for t in (0, 1):
            nc.tensor.matmul(
                ps,
                lhsT=aoT[:, t, st * 128 : (st + 1) * 128],
                rhs=wo_sb[:, t, mb * 512 : (mb + 1) * 512],
                start=(t == 0),
                stop=False,
            )

    def tail_phase2(st, mb, t):
        nc.tensor.matmul(
            tail_ps[(st, mb)],
            lhsT=aoT[:, t, st * 128 : (st + 1) * 128],
            rhs=wo_sb[:, t, mb * 512 : (mb + 1) * 512],
            start=False,
            stop=(t == JT - 1),
        )

    def tail_finish(st, mb, on_vector):
        ps = tail_ps.pop((st, mb))
        if st not in ob_live:
            o_sb = out_pool.tile([128, C], bf16, tag="ob")
            ob_live[st] = o_sb
        o_sb = ob_live[st]
        sl = slice(mb * 512, (mb + 1) * 512)
        if on_vector:
            nc.vector.tensor_copy(out=o_sb[:, sl], in_=ps)
        else:
            nc.scalar.activation(out=o_sb[:, sl], in_=ps, func=Copy)
        if mb == 1:
            nc.sync.dma_start(out=out[st], in_=ob_live.pop(st)[:, :])

    # Per-round fill lists (each entry emits one PSUM group of ~4-9 matmuls).
    fills = {
        0: [
            lambda: proj_k_group(1, 0),
            lambda: proj_k_group(1, 1),
            lambda: proj_q_group(2, 0),
            None,
            lambda: proj_v_group(0),
            lambda: proj_v_group(1),
            lambda: proj_v_group(2),
            lambda: proj_v_group(3),
        ],
        1: [
            lambda: proj_v_group(4),
            lambda: proj_v_group(5),
            lambda: proj_v_group(6),
            lambda: proj_v_group(7),
            lambda: proj_k_group(2, 0),
            lambda: proj_k_group(2, 1),
            lambda: proj_q_group(3, 0),
            None,
        ],
        2: [
            lambda: proj_k_group(3, 0),
            lambda: proj_k_group(3, 1),
            None,
            None,
            None,
            None,
            None,
            None,
        ],
        3: [lambda: proj_q_group(0, 1), lambda: proj_q_group(1, 1),
            None, None, None, None, None, None],
        4: [lambda: proj_q_group(2, 1), None, None, None, None, None, None, None],
        5: [lambda: proj_q_group(3, 1), None, None, None, None, None, None, None],
        6: [
            lambda: out_group(0, 0, on_vector=True),
            None,
            lambda: out_group(0, 1, on_vector=True),
            None,
            None,
            None,
            None,
            None,
        ],
        7: [
            lambda: out_group(1, 0, on_vector=True),
            None,
            lambda: out_group(1, 1, on_vector=True),
            None,
            None,
            None,
            None,
            None,
        ],
    }

    # ---- attention rounds ----
    pt_live = {}      # round -> list of 8 pt tiles
    pend_of = {}      # unit -> o_f tile awaiting the pair's gather
    recip_live = {}   # round -> (o_f_e, o_f_o, rsp_pair)
    norm_live = {}    # round -> (o_f_e, o_f_o, rrow_pair)

    def _dma_eng(r):
        # Tail rounds issue their recip round-trip from the Act engine so the
        # chain doesn't serialize behind everything on the SP issue queue.
        return nc.scalar if r >= NR - 2 else nc.sync

    def finish_pv(r, o_ps, parity):
        # Rowsum row [1,512] -> [128,4] lane spread as ONE direct SBUF->SBUF
        # reshape DMA (no DRAM round trip: each hop costs ~2us of completion
        # latency, which is exposed on the tail's critical path).
        u = 2 * r + parity
        o_f = of_pool.tile([65, 512], f32, tag="of")
        nc.vector.tensor_copy(out=o_f[:, :], in_=o_ps[0:65, :])
        pend_of[u] = o_f
        if parity == 0:
            rsp = rsp_pool.tile([128, 2, 4], f32, tag="rsp")
            pend_of["rsp"] = rsp
        else:
            rsp = pend_of.pop("rsp")
        _dma_eng(r).dma_start(out=rsp[:, parity, :], in_=o_f[64:65, :])
        if parity == 1:
            recip_live[r] = (pend_of.pop(2 * r), pend_of.pop(u), rsp)

    def emit_recip(r):
        o_f_e, o_f_o, rsp = recip_live.pop(r)
        rrow = rrow_pool.tile([65, 1024], bf16, tag="rrow")
        rrp = rrp_pool.tile([128, 2, 4], bf16, tag="rrp")
        with nc.allow_low_precision(reason="bf16 rowsum reciprocal"):
            nc.vector.reciprocal(out=rrp[:, :, :], in_=rsp[:, :, :])
        for u in range(2):
            _dma_eng(r).dma_start(
                out=rrow[64:65, u * 512 : (u + 1) * 512], in_=rrp[:, u, :]
            )
        norm_live[r] = (o_f_e, o_f_o, rrow)

    def emit_norm(r, rb_pool=None):
        qh, p = divmod(r, 4)
        q_sl = slice(qh * 512, (qh + 1) * 512)
        o_f_e, o_f_o, rrow = norm_live.pop(r)
        for parity in range(2):
            o_f = o_f_e if parity == 0 else o_f_o
            if rb_pool is None:
                rb = sp.tile([128, 512], f32, tag="st")
            else:
                rb = rb_pool.tile([128, 512], f32, tag="pv")
            nc.tensor.matmul(
                rb[:, :],
                lhsT=ones_p64[64:65, :],
                rhs=rrow[64:65, parity * 512 : (parity + 1) * 512],
                start=True,
                stop=True,
            )
            if parity == 0:
                nc.vector.tensor_mul(
                    out=aoT[0:64, p, q_sl], in0=o_f[0:64, :], in1=rb[0:64, :]
                )
            else:
                ao_stage = ao_pool.tile([64, 512], bf16, tag="ao")
                nc.vector.tensor_mul(
                    out=ao_stage[:, :], in0=o_f[0:64, :], in1=rb[0:64, :]
                )
                nc.sync.dma_start(out=aoT[64:128, p, q_sl], in_=ao_stage[:, :])

    def emit_round(r):
        """Scores for round r's pair; PV of round r-1; lagged recip/norm."""
        qh, p = divmod(r, 4) if r < NR else (None, None)
        prev = r - 1
        o_e = o_o = None
        if 0 <= prev:
            pqh, pp = divmod(prev, 4)
            o_e = pvp.tile([65, 512], f32, tag="pv")
            o_o = pvp.tile([65, 512], f32, tag="pv")
            pv_tiles = pt_live.get(prev)
        rfills = fills.get(r, [None] * 8)
        pt_tiles = []
        for i in range(SKT):
            if i < len(rfills) and rfills[i] is not None:
                rfills[i]()
            if r < NR:
                q_sl = slice(qh * 512, (qh + 1) * 512)
                S_i = sp.tile([128, 2, 512], f32, tag="st")
                nc.tensor.matmul(
                    S_i[:, 0, :],
                    lhsT=kT[0:64, p, i * 128 : (i + 1) * 128],
                    rhs=qT[0:64, p, q_sl],
                    start=True,
                    stop=True,
                )
                nc.tensor.matmul(
                    S_i[:, 1, :],
                    lhsT=kT[64:128, p, i * 128 : (i + 1) * 128],
                    rhs=qT[64:128, p, q_sl],
                    start=True,
                    stop=True,
                )
            if o_e is not None:
                nc.tensor.matmul(
                    o_e[:, :],
                    lhsT=v_sb[:, i, 2 * pp, :],
                    rhs=pv_tiles[i][:, 0, :],
                    start=(i == 0),
                    stop=(i == SKT - 1),
                )
                nc.tensor.matmul(
                    o_o[:, :],
                    lhsT=v_sb[:, i, 2 * pp + 1, :],
                    rhs=pv_tiles[i][:, 1, :],
                    start=(i == 0),
                    stop=(i == SKT - 1),
                )
            if i == 3 and r - 2 >= 0:
                emit_recip(r - 2)
            if r < NR:
                p_t = pt_pool.tile([128, 2, 512], bf16, tag="pt")
                nc.scalar.activation(
                    out=p_t[:, :, :], in_=S_i[:, :, :], func=Exp, scale=SCALE
                )
                pt_tiles.append(p_t)
        if r < NR:
            pt_live[r] = pt_tiles
        if o_e is not None:
            pt_live.pop(prev)
            finish_pv(prev, o_e, 0)
            finish_pv(prev, o_o, 1)
        if 0 <= r - 2 <= NR - 3:
            emit_norm(r - 2)

    # ---- HAM warmup: dummy matmuls on resident garbage while input DMAs
    # stream in, so the first real matmuls run at 2.4 GHz (the PE clock gate
    # needs ~3.4us of sustained activity to open) ----
    warm_in = persist.tile([128, 512], bf16)
    nc.vector.memset(warm_in[:, :], 1.0)
    warm_ps = pj.tile([128, 512], f32, tag="pj")
    for w in range(24):
        nc.tensor.matmul(
            warm_ps[:, :],
            lhsT=warm_in[:, 0:128],
            rhs=warm_in[:, :],
            start=True,
            stop=True,
        )

    # ---- phase A: Q proj jt0/jt1 (q-half 0) + K jt0 upfront ----
    proj_q_group(0, 0)
    proj_q_group(1, 0)
    proj_k_group(0, 0)
    proj_k_group(0, 1)

    # ---- rounds ----
    for r in range(NR):
        emit_round(r)

    # ---- tail ----
    # PE inventory (PV(7) drain, six t0/t1-partial out groups in st/pj slots,
    # and the four remaining qh0 out groups) is laid out to cover the two
    # recip-chain latencies; only t2/t3 and st7 wait on the final norms.
    g_st0 = sp.tile([128, 2, 512], f32, tag="st")
    g_st1 = sp.tile([128, 2, 512], f32, tag="st")
    fills[NR] = [
        lambda: tail_phase1(4, 0, g_st0[:, 0, :]),
        lambda: tail_phase1(4, 1, g_st0[:, 1, :]),
        lambda: tail_phase1(5, 0, g_st1[:, 0, :]),
        lambda: tail_phase1(5, 1, g_st1[:, 1, :]),
        lambda: out_group(2, 0, on_vector=True),
        lambda: out_group(2, 1, on_vector=True),
        lambda: out_group(3, 0, on_vector=True),
        lambda: out_group(3, 1, on_vector=True),
    ]
    emit_round(NR)         # pv(7) drain + recip(6) + phase1/out fills
    emit_norm(NR - 2, rb_pool=pvp)
    for st, mb in [(4, 0), (4, 1), (5, 0), (5, 1)]:
        tail_phase2(st, mb, 2)
    emit_recip(NR - 1)
    emit_norm(NR - 1, rb_pool=pvp)
    for st, mb in [(4, 0), (4, 1), (5, 0), (5, 1)]:
        tail_phase2(st, mb, 3)
        tail_finish(st, mb, on_vector=(mb == 0))
    out_group(6, 0)
    out_group(6, 1, on_vector=True)
    out_group(7, 0)
    out_group(7, 1, on_vector=True)


def _build():
    import concourse.tile as tile
    from concourse import bacc

    from contextlib import ExitStack

    nc = bacc.Bacc(
        "TRN2", target_bir_lowering=False, debug=False, num_devices=NCORES
    )
    with tile.TileContext(nc) as tc:
        with ExitStack() as ctx:
            _emit(tc, ctx)
    nc.compile()
    return nc


def _get_nc():
    if "nc" not in _CACHED:
        _CACHED["nc"] = _build()
    return _CACHED["nc"]


def _chunk(xT, half):
    """[1024, 1024] -> [128, CT, 512] pretile of columns half*512:..."""
    return np.ascontiguousarray(
        xT.reshape(CT, 128, 2, 512)[:, :, half, :].transpose(1, 0, 2)
    )


def _wtile(w):
    """[1024, 512] -> [128, CT, 512]"""
    return np.ascontiguousarray(w.reshape(CT, 128, HD).transpose(1, 0, 2))


def build_in_maps(inputs):
    import ml_dtypes

    bf = ml_dtypes.bfloat16
    f = np.asarray
    queries = f(inputs["queries"], dtype=np.float32)
    keys = f(inputs["keys"], dtype=np.float32)
    values = f(inputs["values"], dtype=np.float32)
    Wq = f(inputs["Wq"], dtype=np.float32)
    Wk = f(inputs["Wk"], dtype=np.float32)
    Wv = f(inputs["Wv"], dtype=np.float32)
    Wo = f(inputs["Wo"], dtype=np.float32)
    bq = f(inputs["bq"], dtype=np.float32)
    bk = f(inputs["bk"], dtype=np.float32)
    bv = f(inputs["bv"], dtype=np.float32)
    in_maps = []
    for c in range(NCORES):
        b, hh = c // 2, c % 2
        cs = slice(hh * HD, (hh + 1) * HD)
        xqT = queries[b].T.astype(bf)
        xkT = keys[b].T.astype(bf)
        xvT = values[b].T.astype(bf)
        wo_c = Wo[cs, :].astype(bf)
        in_maps.append(
            {
                "xq0": _chunk(xqT, 0),
                "xq1": _chunk(xqT, 1),
                "xk0": _chunk(xkT, 0),
                "xk1": _chunk(xkT, 1),
                "xv0": _chunk(xvT, 0),
                "xv1": _chunk(xvT, 1),
                "wq": _wtile(Wq[:, cs].astype(bf)),
                "wk": _wtile(Wk[:, cs].astype(bf)),
                "wv": _wtile(Wv[:, cs].astype(bf)),
                "wo": np.ascontiguousarray(
                    wo_c.reshape(JT, 128, C).transpose(1, 0, 2)
                ),
                "bq": np.ascontiguousarray(bq[cs].reshape(JT, 128).T),
                "bk": np.ascontiguousarray(bk[cs].reshape(JT, 128).T),
                "bv": np.ascontiguousarray(bv[cs].astype(bf).reshape(1, HD)),
            }
        )
    return in_maps


def kernel(**inputs):
    from concourse.bass_utils import run_bass_kernel_spmd

    nc = _get_nc()
    in_maps = build_in_maps(inputs)
    _CACHED["in_maps"] = in_maps
    res = run_bass_kernel_spmd(nc, in_maps, list(range(NCORES)))
    bo = np.asarray(inputs["bo"], dtype=np.float32)
    full = np.empty((B, S, C), dtype=np.float32)
    for b in range(B):
        p0 = res.results[2 * b]["out"].reshape(S, C).astype(np.float32)
        p1 = res.results[2 * b + 1]["out"].reshape(S, C).astype(np.float32)
        full[b] = p0 + p1 + bo
    return full


# revision 28
# speedup vs baseline: 1.1868x; 1.0457x over previous
"""Trainium2 Bass kernel for multi-head attention (B=4, S=1024, D=1024, H=16).

Sharding: 8 cores = batch(4) x head-half(2). Each core projects Q/K/V for its
8 heads over the full 1024 queries/keys of its batch, runs attention, and
computes a PARTIAL output projection (its heads' contribution to all 1024
output columns). The host sums the two bf16 partials per batch in fp32 and
adds bo -- the "all-reduce after the output projection" is a free host-side
pair-sum. This removes the duplicated K/V projections of a query-split
sharding (-25% MACs).

Attention runs in 8 rounds = (q-half 2) x (head-pair 4). Within a round the
two heads of a pair occupy opposite 64-row strips of the PE array (head 2p at
partitions 0:64, head 2p+1 at 64:128), so their K=64 score matmuls execute
CONCURRENTLY via row tiling (tile_position auto-derived from base_partition).
PV matmuls of the previous round's pair interleave between score pairs, and
projection / output-projection groups are spread into the rounds as PE fill
while the exp chain (ScalarE) paces the pipeline.

All rowsum reciprocals go through the DRAM spread ([1,512] -> [128,4]) so the
DVE does 4 lane-parallel elements instead of a 3.3us lane-serial [1,512] op.
Every DMA is HWDGE (SP queue for x/out + per-round traffic, Act queue for
weights); host pretiles all tensors so DMA lines are 8KB contiguous.
"""

import sys

if "/opt/trn_rl_repo" not in sys.path:
    sys.path.insert(0, "/opt/trn_rl_repo")

import numpy as np

B = 4
S = 1024          # sequence (queries and keys)
C = 1024          # d_model
H = 8             # heads per core
D = 64            # head dim
HD = H * D        # 512 features per core
NCORES = 8
SCALE = 0.125     # 1/sqrt(D)

CT = C // 128     # 8 contraction tiles over d_model
JT = HD // 128    # 4 feature tiles
SKT = S // 128    # 8 key tiles
NR = 8            # rounds = 2 q-halves x 4 head pairs

_CACHED = {}


def _emit(tc, ctx):
    import concourse.bass as bass
    from concourse import mybir

    nc = tc.nc
    f32 = mybir.dt.float32
    bf16 = mybir.dt.bfloat16
    Exp = mybir.ActivationFunctionType.Exp
    Copy = mybir.ActivationFunctionType.Copy

    # ---- DRAM I/O (host pretiles everything to [128, ...] contiguous) ----
    xq_d = [
        nc.dram_tensor(f"xq{i}", [128, CT, 512], bf16, kind="ExternalInput").ap()
        for i in range(2)
    ]
    xk_d = [
        nc.dram_tensor(f"xk{i}", [128, CT, 512], bf16, kind="ExternalInput").ap()
        for i in range(2)
    ]
    xv_d = [
        nc.dram_tensor(f"xv{i}", [128, CT, 512], bf16, kind="ExternalInput").ap()
        for i in range(2)
    ]
    wq = nc.dram_tensor("wq", [128, CT, HD], bf16, kind="ExternalInput").ap()
    wk = nc.dram_tensor("wk", [128, CT, HD], bf16, kind="ExternalInput").ap()
    wv = nc.dram_tensor("wv", [128, CT, HD], bf16, kind="ExternalInput").ap()
    wo = nc.dram_tensor("wo", [128, JT, C], bf16, kind="ExternalInput").ap()
    bq = nc.dram_tensor("bq", [128, JT], f32, kind="ExternalInput").ap()
    bk = nc.dram_tensor("bk", [128, JT], f32, kind="ExternalInput").ap()
    bv = nc.dram_tensor("bv", [1, HD], bf16, kind="ExternalInput").ap()
    # out[st] = [128, 1024] rows st*128..st*128+128 of the partial output
    out = nc.dram_tensor("out", [SKT, 128, C], bf16, kind="ExternalOutput").ap()



    # ---- long-lived SBUF ----
    persist = ctx.enter_context(tc.tile_pool(name="persist", bufs=1))
    qT = persist.tile([128, JT, S], bf16)       # [feat, jt, query]
    kT = persist.tile([128, JT, S], bf16)       # [feat, jt, key]
    v_sb = persist.tile([128, SKT, H, D + 1], bf16)
    wo_sb = persist.tile([128, JT, C], bf16)
    aoT = persist.tile([128, JT, S], bf16)
    xq_sb = persist.tile([128, 2, CT, 512], bf16)   # [p, q-half, ct, q]
    xk_sb = persist.tile([128, 2, CT, 512], bf16)   # [p, k-half, ct, k]
    xv_sb = persist.tile([128, 2, CT, 512], bf16)
    wq_sb = persist.tile([128, CT, HD], bf16)
    wk_sb = persist.tile([128, CT, HD], bf16)
    wv_sb = persist.tile([128, CT, HD], bf16)
    bq_col = persist.tile([128, JT], f32)
    bk_col = persist.tile([128, JT], f32)
    bv_row = persist.tile([1, HD], bf16)
    ones_col = persist.tile([1, 128], bf16)
    ones_p64 = persist.tile([65, 128], bf16)

    nc.vector.memset(ones_col[:, :], 1.0)
    nc.vector.memset(ones_p64[:, :], 1.0)
    nc.vector.memset(v_sb[:, :, :, D : D + 1], 1.0)

    # ---- DMA issue: weights on the Act HWDGE queue, x on the SP queue ----
    # wq first on the (faster-starting) SP queue: the first PE work is the
    # q-half-0 projections and they need wq + xq0 before anything else.
    nc.scalar.dma_start(out=bq_col[:, :], in_=bq)
    nc.scalar.dma_start(out=bk_col[:, :], in_=bk)
    nc.scalar.dma_start(out=bv_row[:, :], in_=bv)
    nc.scalar.dma_start(out=wk_sb[:, :, :], in_=wk)
    nc.scalar.dma_start(out=wv_sb[:, :, :], in_=wv)
    nc.scalar.dma_start(out=wo_sb[:, :, :], in_=wo)

    nc.sync.dma_start(out=wq_sb[:, :, :], in_=wq)
    nc.sync.dma_start(out=xq_sb[:, 0], in_=xq_d[0])
    nc.sync.dma_start(out=xk_sb[:, 0], in_=xk_d[0])
    nc.sync.dma_start(out=xk_sb[:, 1], in_=xk_d[1])
    nc.sync.dma_start(out=xv_sb[:, 0], in_=xv_d[0])
    nc.sync.dma_start(out=xv_sb[:, 1], in_=xv_d[1])
    nc.sync.dma_start(out=xq_sb[:, 1], in_=xq_d[1])

    # ---- pools ----
    pj = ctx.enter_context(tc.tile_pool(name="pj_psum", bufs=2, space="PSUM"))
    sp = ctx.enter_context(tc.tile_pool(name="st_psum", bufs=2, space="PSUM"))
    pvp = ctx.enter_context(tc.tile_pool(name="pv_psum", bufs=2, space="PSUM"))
    pt_pool = ctx.enter_context(tc.tile_pool(name="pt", bufs=18))
    of_pool = ctx.enter_context(tc.tile_pool(name="of", bufs=8))
    rrow_pool = ctx.enter_context(tc.tile_pool(name="rrow", bufs=6))
    rsp_pool = ctx.enter_context(tc.tile_pool(name="rsp", bufs=4))
    rrp_pool = ctx.enter_context(tc.tile_pool(name="rrp", bufs=4))
    ao_pool = ctx.enter_context(tc.tile_pool(name="ao_stage", bufs=3))
    out_pool = ctx.enter_context(tc.tile_pool(name="out_sb", bufs=3))

    # ---- projection / output groups (PE fill work) ----
    def proj_q_group(jt, half):
        ps = pj.tile([128, 512], f32, tag="pj")
        for ct in range(CT):
            nc.tensor.matmul(
                ps[:, :],
                lhsT=wq_sb[:, ct, jt * 128 : (jt + 1) * 128],
                rhs=xq_sb[:, half, ct, :],
                start=(ct == 0),
                stop=(ct == CT - 1),
            )
        nc.vector.tensor_scalar_add(
            out=qT[:, jt, half * 512 : (half + 1) * 512],
            in0=ps[:, :],
            scalar1=bq_col[:, jt : jt + 1],
        )

    def proj_k_group(jt, half):
        ps = pj.tile([128, 512], f32, tag="pj")
        for ct in range(CT):
            nc.tensor.matmul(
                ps[:, :],
                lhsT=wk_sb[:, ct, jt * 128 : (jt + 1) * 128],
                rhs=xk_sb[:, half, ct, :],
                start=(ct == 0),
                stop=(ct == CT - 1),
            )
        nc.vector.tensor_scalar_add(
            out=kT[:, jt, half * 512 : (half + 1) * 512],
            in0=ps[:, :],
            scalar1=bk_col[:, jt : jt + 1],
        )

    def proj_v_group(skt):
        kb, ko = skt // 4, skt % 4
        ps = pj.tile([128, 512], f32, tag="pj")
        for ct in range(CT):
            nc.tensor.matmul(
                ps[:, :],
                lhsT=xv_sb[:, kb, ct, ko * 128 : (ko + 1) * 128],
                rhs=wv_sb[:, ct, :],
                start=(ct == 0),
                stop=False,
            )
        nc.tensor.matmul(
            ps[:, :],
            lhsT=ones_col[:, :],
            rhs=bv_row[:, :],
            start=False,
            stop=True,
        )
        nc.vector.tensor_copy(
            out=v_sb[:, skt, :, 0:D],
            in_=ps.rearrange("p (h d) -> p h d", d=D),
        )

    ob_live = {}

    def out_group(st, mb, on_vector=False):
        ps = pj.tile([128, 512], f32, tag="pj")
        for t in range(JT):
            nc.tensor.matmul(
                ps[:, :],
                lhsT=aoT[:, t, st * 128 : (st + 1) * 128],
                rhs=wo_sb[:, t, mb * 512 : (mb + 1) * 512],
                start=(t == 0),
                stop=(t == JT - 1),
            )
        if st not in ob_live:
            o_sb = out_pool.tile([128, C], bf16, tag="ob")
            ob_live[st] = o_sb
        o_sb = ob_live[st]
        sl = slice(mb * 512, (mb + 1) * 512)
        if on_vector:
            nc.vector.tensor_copy(out=o_sb[:, sl], in_=ps[:, :])
        else:
            nc.scalar.activation(out=o_sb[:, sl], in_=ps[:, :], func=Copy)
        if mb == 1:
            nc.sync.dma_start(out=out[st], in_=ob_live.pop(st)[:, :])

    # Tail out-groups for rows 512+: phase 1 accumulates t0/t1 into an open
    # PSUM group during the PV drain; phase 2 adds t2/t3 once the last norms
    # land. Six groups live in st/pj slots; the last two run as plain groups.
    tail_ps = {}

    def tail_phase1(st, mb, ps):
        tail_ps[(st, mb)] = ps
        for t in (0, 1):
            nc.tensor.matmul(
                ps,
                lhsT=aoT[:, t, st * 128 : (st + 1) * 128],
                rhs=wo_sb[:, t, mb * 512 : (mb + 1) * 512],
                start=(t == 0),
                stop=False,
            )

    def tail_phase2(st, mb, t):
        nc.tensor.matmul(
            tail_ps[(st, mb)],
            lhsT=aoT[:, t, st * 128 : (st + 1) * 128],
            rhs=wo_sb[:, t, mb * 512 : (mb + 1) * 512],
            start=False,
            stop=(t == JT - 1),
        )

    def tail_finish(st, mb, on_vector):
        ps = tail_ps.pop((st, mb))
        if st not in ob_live:
            o_sb = out_pool.tile([128, C], bf16, tag="ob")
            ob_live[st] = o_sb
        o_sb = ob_live[st]
        sl = slice(mb * 512, (mb + 1) * 512)
        if on_vector:
            nc.vector.tensor_copy(out=o_sb[:, sl], in_=ps)
        else:
            nc.scalar.activation(out=o_sb[:, sl], in_=ps, func=Copy)
        if mb == 1:
            nc.sync.dma_start(out=out[st], in_=ob_live.pop(st)[:, :])

    # Per-round fill lists (each entry emits one PSUM group of ~4-9 matmuls).
    fills = {
        0: [
            lambda: proj_k_group(1, 0),
            lambda: proj_k_group(1, 1),
            lambda: proj_q_group(2, 0),
            None,
            lambda: proj_v_group(0),
            lambda: proj_v_group(1),
            lambda: proj_v_group(2),
            lambda: proj_v_group(3),
        ],
        1: [
            lambda: proj_v_group(4),
            lambda: proj_v_group(5),
            lambda: proj_v_group(6),
            lambda: proj_v_group(7),
            lambda: proj_k_group(2, 0),
            lambda: proj_k_group(2, 1),
            lambda: proj_q_group(3, 0),
            None,
        ],
        2: [
            lambda: proj_k_group(3, 0),
            lambda: proj_k_group(3, 1),
            None,
            None,
            None,
            None,
            None,
            None,
        ],
        3: [lambda: proj_q_group(0, 1), lambda: proj_q_group(1, 1),
            None, None, None, None, None, None],
        4: [lambda: proj_q_group(2, 1), None, None, None, None, None, None, None],
        5: [lambda: proj_q_group(3, 1), None, None, None, None, None, None, None],
        6: [
            lambda: out_group(0, 0, on_vector=True),
            None,
            lambda: out_group(0, 1, on_vector=True),
            None,
            None,
            None,
            None,
            None,
        ],
        7: [
            lambda: out_group(1, 0, on_vector=True),
            None,
            lambda: out_group(1, 1, on_vector=True),
            None,
            None,
            None,
            None,
            None,
        ],
    }

    # ---- attention rounds ----
    pt_live = {}      # round -> list of 8 pt tiles
    pend_of = {}      # unit -> o_f tile awaiting the pair's gather
    recip_live = {}   # round -> (o_f_e, o_f_o, rsp_pair)
    norm_live = {}    # round -> (o_f_e, o_f_o, rrow_pair)

    def _dma_eng(r):
        # Tail rounds issue their recip round-trip from the Act engine so the
        # chain doesn't serialize behind everything on the SP issue queue.
        return nc.scalar if r >= NR - 2 else nc.sync

    def finish_pv(r, o_ps, parity):
        # Rowsum row [1,512] -> [128,4] lane spread as ONE direct SBUF->SBUF
        # reshape DMA (no DRAM round trip: each hop costs ~2us of completion
        # latency, which is exposed on the tail's critical path).
        u = 2 * r + parity
        o_f = of_pool.tile([65, 512], f32, tag="of")
        nc.vector.tensor_copy(out=o_f[:, :], in_=o_ps[0:65, :])
        pend_of[u] = o_f
        if parity == 0:
            rsp = rsp_pool.tile([128, 2, 4], f32, tag="rsp")
            pend_of["rsp"] = rsp
        else:
            rsp = pend_of.pop("rsp")
        _dma_eng(r).dma_start(out=rsp[:, parity, :], in_=o_f[64:65, :])
        if parity == 1:
            recip_live[r] = (pend_of.pop(2 * r), pend_of.pop(u), rsp)

    def emit_recip(r):
        o_f_e, o_f_o, rsp = recip_live.pop(r)
        rrow = rrow_pool.tile([65, 1024], bf16, tag="rrow")
        rrp = rrp_pool.tile([128, 2, 4], bf16, tag="rrp")
        with nc.allow_low_precision(reason="bf16 rowsum reciprocal"):
            nc.vector.reciprocal(out=rrp[:, :, :], in_=rsp[:, :, :])
        for u in range(2):
            _dma_eng(r).dma_start(
                out=rrow[64:65, u * 512 : (u + 1) * 512], in_=rrp[:, u, :]
            )
        norm_live[r] = (o_f_e, o_f_o, rrow)

    def emit_norm(r, rb_pool=None):
        qh, p = divmod(r, 4)
        q_sl = slice(qh * 512, (qh + 1) * 512)
        o_f_e, o_f_o, rrow = norm_live.pop(r)
        for parity in (1, 0):  # odd first: its aoT half goes through a DMA
            o_f = o_f_e if parity == 0 else o_f_o
            if rb_pool is None:
                rb = sp.tile([128, 512], f32, tag="st")
            else:
                rb = rb_pool.tile([128, 512], f32, tag="pv")
            nc.tensor.matmul(
                rb[:, :],
                lhsT=ones_p64[64:65, :],
                rhs=rrow[64:65, parity * 512 : (parity + 1) * 512],
                start=True,
                stop=True,
            )
            if parity == 0:
                nc.vector.tensor_mul(
                    out=aoT[0:64, p, q_sl], in0=o_f[0:64, :], in1=rb[0:64, :]
                )
            else:
                ao_stage = ao_pool.tile([64, 512], bf16, tag="ao")
                nc.vector.tensor_mul(
                    out=ao_stage[:, :], in0=o_f[0:64, :], in1=rb[0:64, :]
                )
                nc.sync.dma_start(out=aoT[64:128, p, q_sl], in_=ao_stage[:, :])

    def emit_round(r):
        """Scores for round r's pair; PV of round r-1; lagged recip/norm."""
        qh, p = divmod(r, 4) if r < NR else (None, None)
        prev = r - 1
        o_e = o_o = None
        if 0 <= prev:
            pqh, pp = divmod(prev, 4)
            o_e = pvp.tile([65, 512], f32, tag="pv")
            o_o = pvp.tile([65, 512], f32, tag="pv")
            pv_tiles = pt_live.get(prev)
        rfills = fills.get(r, [None] * 8)
        pt_tiles = []
        # PV of the previous pair runs DENSE in the first half of the round
        # (4 matmuls per slot) so finish_pv + the recip chain start mid-round
        # and the reciprocal is ready well before its norm. Round 1 keeps the
        # spread layout (2/slot): its V projections are still streaming in.
        dense_pv = o_e is not None and prev >= 1
        for i in range(SKT):
            if i < len(rfills) and rfills[i] is not None:
                rfills[i]()
            if r < NR:
                q_sl = slice(qh * 512, (qh + 1) * 512)
                S_i = sp.tile([128, 2, 512], f32, tag="st")
                nc.tensor.matmul(
                    S_i[:, 0, :],
                    lhsT=kT[0:64, p, i * 128 : (i + 1) * 128],
                    rhs=qT[0:64, p, q_sl],
                    start=True,
                    stop=True,
                )
                nc.tensor.matmul(
                    S_i[:, 1, :],
                    lhsT=kT[64:128, p, i * 128 : (i + 1) * 128],
                    rhs=qT[64:128, p, q_sl],
                    start=True,
                    stop=True,
                )
            if o_e is not None:
                sks = []
                if dense_pv and i < 4:
                    sks = [2 * i, 2 * i + 1]
                elif not dense_pv:
                    sks = [i]
                for sk in sks:
                    nc.tensor.matmul(
                        o_e[:, :],
                        lhsT=v_sb[:, sk, 2 * pp, :],
                        rhs=pv_tiles[sk][:, 0, :],
                        start=(sk == 0),
                        stop=(sk == SKT - 1),
                    )
                    nc.tensor.matmul(
                        o_o[:, :],
                        lhsT=v_sb[:, sk, 2 * pp + 1, :],
                        rhs=pv_tiles[sk][:, 1, :],
                        start=(sk == 0),
                        stop=(sk == SKT - 1),
                    )
            if i == 3 and o_e is not None and dense_pv:
                pt_live.pop(prev)
                finish_pv(prev, o_e, 0)
                finish_pv(prev, o_o, 1)
            if i == 3 and (r - 2) in recip_live:
                emit_recip(r - 2)
            if r < NR:
                p_t = pt_pool.tile([128, 2, 512], bf16, tag="pt")
                nc.scalar.activation(
                    out=p_t[:, :, :], in_=S_i[:, :, :], func=Exp, scale=SCALE
                )
                pt_tiles.append(p_t)
        if r < NR:
            pt_live[r] = pt_tiles
        if o_e is not None and not dense_pv:
            pt_live.pop(prev)
            finish_pv(prev, o_e, 0)
            finish_pv(prev, o_o, 1)
        if dense_pv and prev in recip_live:
            emit_recip(prev)
        if 0 <= r - 2 <= NR - 3 and (r - 2) in norm_live:
            emit_norm(r - 2)

    # ---- HAM warmup: dummy matmuls on resident garbage while input DMAs
    # stream in, so the first real matmuls run at 2.4 GHz (the PE clock gate
    # needs ~3.4us of sustained activity to open) ----
    warm_in = persist.tile([128, 512], bf16)
    nc.vector.memset(warm_in[:, :], 1.0)
    warm_ps = pj.tile([128, 512], f32, tag="pj")
    for w in range(24):
        nc.tensor.matmul(
            warm_ps[:, :],
            lhsT=warm_in[:, 0:128],
            rhs=warm_in[:, :],
            start=True,
            stop=True,
        )

    # ---- phase A: Q proj jt0/jt1 (q-half 0) + K jt0 upfront ----
    proj_q_group(0, 0)
    proj_q_group(1, 0)
    proj_k_group(0, 0)
    proj_k_group(0, 1)

    # ---- rounds ----
    for r in range(NR):
        emit_round(r)

    # ---- tail ----
    # PE inventory (PV(7) drain, six t0/t1-partial out groups in st/pj slots,
    # and the four remaining qh0 out groups) is laid out to cover the two
    # recip-chain latencies; only t2/t3 and st7 wait on the final norms.
    g_st0 = sp.tile([128, 2, 512], f32, tag="st")
    g_st1 = sp.tile([128, 2, 512], f32, tag="st")
    fills[NR] = [
        lambda: tail_phase1(4, 0, g_st0[:, 0, :]),
        lambda: tail_phase1(4, 1, g_st0[:, 1, :]),
        lambda: tail_phase1(5, 0, g_st1[:, 0, :]),
        lambda: tail_phase1(5, 1, g_st1[:, 1, :]),
        lambda: out_group(2, 0, on_vector=True),
        lambda: out_group(2, 1, on_vector=True),
        lambda: out_group(3, 0, on_vector=True),
        lambda: out_group(3, 1, on_vector=True),
    ]
    emit_round(NR)         # pv(7) dense + finish(7) + recip(7) + phase1/out fills
    emit_norm(NR - 2, rb_pool=pvp)
    for st, mb in [(4, 0), (4, 1), (5, 0), (5, 1)]:
        tail_phase2(st, mb, 2)
    emit_norm(NR - 1, rb_pool=pvp)
    for st, mb in [(4, 0), (4, 1), (5, 0), (5, 1)]:
        tail_phase2(st, mb, 3)
        tail_finish(st, mb, on_vector=(mb == 0))
    out_group(6, 0)
    out_group(6, 1, on_vector=True)
    out_group(7, 0)
    out_group(7, 1, on_vector=True)


def _build():
    import concourse.tile as tile
    from concourse import bacc

    from contextlib import ExitStack

    nc = bacc.Bacc(
        "TRN2", target_bir_lowering=False, debug=False, num_devices=NCORES
    )
    with tile.TileContext(nc) as tc:
        with ExitStack() as ctx:
            _emit(tc, ctx)
    nc.compile()
    return nc


def _get_nc():
    if "nc" not in _CACHED:
        _CACHED["nc"] = _build()
    return _CACHED["nc"]


def _chunk(xT, half):
    """[1024, 1024] -> [128, CT, 512] pretile of columns half*512:..."""
    return np.ascontiguousarray(
        xT.reshape(CT, 128, 2, 512)[:, :, half, :].transpose(1, 0, 2)
    )


def _wtile(w):
    """[1024, 512] -> [128, CT, 512]"""
    return np.ascontiguousarray(w.reshape(CT, 128, HD).transpose(1, 0, 2))


def build_in_maps(inputs):
    import ml_dtypes

    bf = ml_dtypes.bfloat16
    f = np.asarray
    queries = f(inputs["queries"], dtype=np.float32)
    keys = f(inputs["keys"], dtype=np.float32)
    values = f(inputs["values"], dtype=np.float32)
    Wq = f(inputs["Wq"], dtype=np.float32)
    Wk = f(inputs["Wk"], dtype=np.float32)
    Wv = f(inputs["Wv"], dtype=np.float32)
    Wo = f(inputs["Wo"], dtype=np.float32)
    bq = f(inputs["bq"], dtype=np.float32)
    bk = f(inputs["bk"], dtype=np.float32)
    bv = f(inputs["bv"], dtype=np.float32)
    in_maps = []
    for c in range(NCORES):
        b, hh = c // 2, c % 2
        cs = slice(hh * HD, (hh + 1) * HD)
        xqT = queries[b].T.astype(bf)
        xkT = keys[b].T.astype(bf)
        xvT = values[b].T.astype(bf)
        wo_c = Wo[cs, :].astype(bf)
        in_maps.append(
            {
                "xq0": _chunk(xqT, 0),
                "xq1": _chunk(xqT, 1),
                "xk0": _chunk(xkT, 0),
                "xk1": _chunk(xkT, 1),
                "xv0": _chunk(xvT, 0),
                "xv1": _chunk(xvT, 1),
                "wq": _wtile(Wq[:, cs].astype(bf)),
                "wk": _wtile(Wk[:, cs].astype(bf)),
                "wv": _wtile(Wv[:, cs].astype(bf)),
                "wo": np.ascontiguousarray(
                    wo_c.reshape(JT, 128, C).transpose(1, 0, 2)
                ),
                "bq": np.ascontiguousarray(bq[cs].reshape(JT, 128).T),
                "bk": np.ascontiguousarray(bk[cs].reshape(JT, 128).T),
                "bv": np.ascontiguousarray(bv[cs].astype(bf).reshape(1, HD)),
            }
        )
    return in_maps


def kernel(**inputs):
    from concourse.bass_utils import run_bass_kernel_spmd

    nc = _get_nc()
    in_maps = build_in_maps(inputs)
    _CACHED["in_maps"] = in_maps
    res = run_bass_kernel_spmd(nc, in_maps, list(range(NCORES)))
    bo = np.asarray(inputs["bo"], dtype=np.float32)
    full = np.empty((B, S, C), dtype=np.float32)
    for b in range(B):
        p0 = res.results[2 * b]["out"].reshape(S, C).astype(np.float32)
        p1 = res.results[2 * b + 1]["out"].reshape(S, C).astype(np.float32)
        full[b] = p0 + p1 + bo
    return full
